# revision 38
# baseline (speedup 1.0000x reference)
"""Trainium2 Bass kernel for nn_DeltaNet_19430432047178.

Strategy (8 cores, SPMD):
  - activations live transposed on device: [d, T] with d on partitions
  - depthwise conv k=3 computed on the vector engine (3 shifted
    per-partition FMAs); pointwise conv is a single row-sharded matmul
  - attention tensor-parallel over heads (2 heads/core), chunked linear
    attention (C=128) with KV state accumulated in PSUM
  - w_o / delta-gate / LN row-sharded over d
  - MoE sharded expert x hidden-half (core c -> expert c//2, half c%2)
  - all collectives are chunked over T (4 chunks of 512) and pipelined
    under compute: 4x AG(x1), 4x AG(attn), 4x AR(LN stats), 4x AG(h),
    4x RS(ffn partial, bf16)
  - gpsimd issues only collectives; data DMAs ride sync/scalar (HWDGE)
  - all matmuls bf16 with f32 PSUM accumulation
"""
import numpy as np
import ml_dtypes


NC_N = 8
T = 2048
D = 1024
H = 16
DH = 64
E = 4
HD = 4096
P = 128
CH = 128            # attention chunk
NCH = T // CH       # 16
NT = 4              # T chunks of 512 for GEMMs and collectives
TC = 512

BF = ml_dtypes.bfloat16

_PROGRAM = None  # cached nc


def _build_program():
    import concourse.mybir as mybir
    import concourse.tile as tile
    from concourse import bacc
    from concourse.masks import make_identity

    f32 = mybir.dt.float32
    bf16 = mybir.dt.bfloat16
    AF = mybir.ActivationFunctionType
    OP = mybir.AluOpType

    nc = bacc.Bacc()

    # ---------------- external params (per-core) ----------------
    xtp_d = nc.declare_dram_parameter("xtp", [D, T + 2], bf16, isOutput=False)
    xs32_d = nc.declare_dram_parameter("xs32", [P, T], f32, isOutput=False)
    wdw_d = nc.declare_dram_parameter("wdw", [P, 24], f32, isOutput=False)
    wpw_d = nc.declare_dram_parameter("wpw", [P, 64 * P], bf16, isOutput=False)
    wqkv_d = nc.declare_dram_parameter("wqkv", [P, 8 * 384], bf16, isOutput=False)
    ctab_d = nc.declare_dram_parameter("ctab", [P, T], bf16, isOutput=False)
    stab_d = nc.declare_dram_parameter("stab", [P, T], bf16, isOutput=False)
    p64_d = nc.declare_dram_parameter("p64", [P, P], bf16, isOutput=False)
    mask_d = nc.declare_dram_parameter("mask", [P, P], f32, isOutput=False)
    wo_d = nc.declare_dram_parameter("wo", [P, 8 * P], bf16, isOutput=False)
    wg_d = nc.declare_dram_parameter("wg", [P, 8 * P], bf16, isOutput=False)
    bg_d = nc.declare_dram_parameter("bg", [P, 1], f32, isOutput=False)
    lng_d = nc.declare_dram_parameter("lng", [P, 1], f32, isOutput=False)
    lnb_d = nc.declare_dram_parameter("lnb", [P, 1], f32, isOutput=False)
    wmg_d = nc.declare_dram_parameter("wmg", [P, 8 * 4], bf16, isOutput=False)
    esel_d = nc.declare_dram_parameter("esel", [4, 1], bf16, isOutput=False)
    w1s_d = nc.declare_dram_parameter("w1s", [P, 8 * 2048], bf16, isOutput=False)
    b1s_d = nc.declare_dram_parameter("b1s", [P, 16], f32, isOutput=False)
    w2s_d = nc.declare_dram_parameter("w2s", [P, 16 * 1024], bf16, isOutput=False)
    b2s_d = nc.declare_dram_parameter("b2s", [1, 1024], bf16, isOutput=False)
    y_d = nc.declare_dram_parameter("y", [P, T], f32, isOutput=True)

    # ---------------- internal DRAM (chunked collectives) ----------------
    rg = [list(range(NC_N))]
    dum_in = nc.dram_tensor("dum_in", [P, 16], bf16)
    dum_out = nc.dram_tensor("dum_out", [D, 16], bf16, addr_space="Shared")
    ag2_in = [nc.dram_tensor(f"ag2_in{t}", [P, TC], bf16) for t in range(NT)]
    ag2_out = [nc.dram_tensor(f"ag2_out{t}", [D, TC], bf16, addr_space="Shared")
               for t in range(NT)]
    ar_in = [nc.dram_tensor(f"ar_in{t}", [2, TC], f32) for t in range(NT)]
    ar_out = [nc.dram_tensor(f"ar_out{t}", [2, TC], f32, addr_space="Shared")
              for t in range(NT)]
    ag3_in = [nc.dram_tensor(f"ag3_in{t}", [P, TC], bf16) for t in range(NT)]
    ag3_out = [nc.dram_tensor(f"ag3_out{t}", [D, TC], bf16, addr_space="Shared")
               for t in range(NT)]
    rs_in = [nc.dram_tensor(f"rs_in{t}", [D, TC], bf16) for t in range(NT)]
    rs_out = [nc.dram_tensor(f"rs_out{t}", [P, TC], bf16) for t in range(NT)]

    import concourse.bass as bass
    import contextlib

    with tile.TileContext(nc) as tc:
        with contextlib.ExitStack() as stack:
            consts = stack.enter_context(tc.tile_pool(name="consts", bufs=1))

            # ---- constants / weights ----
            wdw_sb = consts.tile([P, 24], f32, tag="wdw")
            nc.scalar.dma_start(out=wdw_sb[:], in_=wdw_d[:])
            wpw_sb = consts.tile([P, 64 * P], bf16, tag="wpw")
            nc.scalar.dma_start(out=wpw_sb[:, 0:32 * P], in_=wpw_d[:, 0:32 * P])
            nc.scalar.dma_start(out=wpw_sb[:, 32 * P:], in_=wpw_d[:, 32 * P:])
            wqkv_sb = consts.tile([P, 8 * 384], bf16, tag="wqkv")
            nc.scalar.dma_start(out=wqkv_sb[:], in_=wqkv_d[:])
            wo_sb = consts.tile([P, 8 * P], bf16, tag="wo")
            nc.scalar.dma_start(out=wo_sb[:], in_=wo_d[:])
            wg_sb = consts.tile([P, 8 * P], bf16, tag="wg")
            nc.scalar.dma_start(out=wg_sb[:], in_=wg_d[:])
            bg_sb = consts.tile([P, 1], f32, tag="bg")
            nc.scalar.dma_start(out=bg_sb[:], in_=bg_d[:])
            lng_sb = consts.tile([P, 1], f32, tag="lng")
            nc.scalar.dma_start(out=lng_sb[:], in_=lng_d[:])
            lnb_sb = consts.tile([P, 1], f32, tag="lnb")
            nc.scalar.dma_start(out=lnb_sb[:], in_=lnb_d[:])
            wmg_sb = consts.tile([P, 8 * 4], bf16, tag="wmg")
            nc.scalar.dma_start(out=wmg_sb[:], in_=wmg_d[:])
            esel_sb = consts.tile([4, 1], bf16, tag="esel")
            nc.scalar.dma_start(out=esel_sb[:], in_=esel_d[:])
            b1s_sb = consts.tile([P, 16], f32, tag="b1s")
            nc.scalar.dma_start(out=b1s_sb[:], in_=b1s_d[:])
            b2s_sb = consts.tile([1, 1024], bf16, tag="b2s")
            nc.scalar.dma_start(out=b2s_sb[:], in_=b2s_d[:])
            ctab_sb = consts.tile([P, T], bf16, tag="ctab")
            nc.scalar.dma_start(out=ctab_sb[:], in_=ctab_d[:])
            stab_sb = consts.tile([P, T], bf16, tag="stab")
            nc.scalar.dma_start(out=stab_sb[:], in_=stab_d[:])
            p64_sb = consts.tile([P, P], bf16, tag="p64")
            nc.scalar.dma_start(out=p64_sb[:], in_=p64_d[:])
            mask_sb = consts.tile([P, P], f32, tag="mask")
            nc.scalar.dma_start(out=mask_sb[:], in_=mask_d[:])
            ident_sb = consts.tile([P, P], bf16, tag="ident")
            make_identity(nc, ident_sb[:])
            ones128 = consts.tile([P, 1], bf16, tag="ones128")
            nc.vector.memset(ones128[:], 1.0)
            ones4 = consts.tile([4, 1], bf16, tag="ones4")
            nc.vector.memset(ones4[:], 1.0)
            ones512 = consts.tile([1, TC], bf16, tag="ones512")
            nc.vector.memset(ones512[:], 1.0)
            ones1f = consts.tile([1, P], f32, tag="ones1f")
            nc.vector.memset(ones1f[:], 1.0)
            eps128 = consts.tile([P, 1], f32, tag="eps128")
            nc.vector.memset(eps128[:], 1e-5)

            # warm up collectives (barrier + first-RDH cost) while DMAs run
            dum_sb = consts.tile([P, 16], bf16, tag="dum")
            nc.vector.memset(dum_sb[:], 0.0)
            nc.scalar.dma_start(out=dum_in[:], in_=dum_sb[:])
            nc.gpsimd.collective_compute(
                "AllGather", mybir.AluOpType.bypass, replica_groups=rg,
                ins=[dum_in[:]], outs=[dum_out[:]])

            # MoE weights prefetch (sync queue; needed from ~mid-kernel)
            w1_sb = [consts.tile([P, 2048], bf16, tag=f"w1_{k}", name=f"w1_{k}")
                     for k in range(8)]
            for k in range(8):
                nc.sync.dma_start(out=w1_sb[k][:],
                                  in_=w1s_d[:, k * 2048:(k + 1) * 2048])
            w2_sb = [consts.tile([P, 1024], bf16, tag=f"w2_{k}", name=f"w2_{k}")
                     for k in range(16)]
            for k in range(16):
                nc.sync.dma_start(out=w2_sb[k][:],
                                  in_=w2s_d[:, k * 1024:(k + 1) * 1024])

            # persistent activations
            late_stack = contextlib.ExitStack()
            late = late_stack.enter_context(tc.tile_pool(name="late", bufs=1))
            x2s32 = late.tile([P, T], f32, tag="x2s32")
            x1f = [late.tile([P, T], bf16, tag=f"x1f{i}", name=f"x1f{i}")
                   for i in range(8)]
            wx1_stack = contextlib.ExitStack()
            wx1 = wx1_stack.enter_context(tc.tile_pool(name="wx1", bufs=1))
            x1s32 = wx1.tile([P, T], f32, tag="x1s32")
            g_sb = wx1.tile([P, T + 1], bf16, tag="g_sb")

            # =================== Phase A: depthwise (DVE) + pointwise ====
            xtp_stack = contextlib.ExitStack()
            xtp_pool = xtp_stack.enter_context(tc.tile_pool(name="xtp", bufs=1))
            xtp = [xtp_pool.tile([P, T + 2], bf16, tag=f"xtp{i}", name=f"xtp{i}")
                   for i in range(8)]
            for k in range(8):
                eng = nc.scalar if k % 2 == 0 else nc.sync
                eng.dma_start(out=xtp[k][:], in_=xtp_d[k * P:(k + 1) * P, :])
            # conv is computed replicated: every core computes all 8 output
            # blocks of x1 (no AllGather). The block order is rotated
            # per-core in the host packing so block i=0 is this core's own
            # 128 rows (wqkv/wg k-blocks are packed with the same rotation).
            with tc.tile_pool(name="pa", bufs=2) as pa, \
                 tc.tile_pool(name="dwy", bufs=1) as dwy_pool, \
                 tc.tile_pool(name="pa_ps", bufs=3, space="PSUM") as pa_ps:
                for t in range(NT):
                    sl = slice(t * TC, (t + 1) * TC)
                    xs32_sb = pa.tile([P, TC], f32, tag="xs32")
                    nc.scalar.dma_start(out=xs32_sb[:], in_=xs32_d[:, sl])
                    dk = []
                    for k in range(8):
                        # dwy = sum_j xtp[k][:, t*TC+j : +TC] * wdw[:, 3k+j]
                        d = dwy_pool.tile([P, TC], bf16, tag=f"dwy{k}")
                        nc.vector.tensor_scalar_mul(
                            d[:], xtp[k][:, t * TC:t * TC + TC],
                            wdw_sb[:, 3 * k:3 * k + 1])
                        nc.vector.scalar_tensor_tensor(
                            out=d[:], in0=xtp[k][:, t * TC + 1:t * TC + 1 + TC],
                            scalar=wdw_sb[:, 3 * k + 1:3 * k + 2], in1=d[:],
                            op0=OP.mult, op1=OP.add)
                        nc.vector.scalar_tensor_tensor(
                            out=d[:], in0=xtp[k][:, t * TC + 2:t * TC + 2 + TC],
                            scalar=wdw_sb[:, 3 * k + 2:3 * k + 3], in1=d[:],
                            op0=OP.mult, op1=OP.add)
                        dk.append(d)
                    for i in range(8):
                        ps = pa_ps.tile([P, TC], f32, tag="mm")
                        for k in range(8):
                            nc.tensor.matmul(
                                ps[:],
                                lhsT=wpw_sb[:, (k * 8 + i) * P:(k * 8 + i + 1) * P],
                                rhs=dk[k][:],
                                start=(k == 0), stop=(k == 7))
                        # x1 = x + y (xtp rows are rotated like the out-blocks)
                        nc.vector.tensor_add(
                            x1f[i][:, sl], xtp[i][:, t * TC + 2:t * TC + 2 + TC],
                            ps[:])
                        if i == 0:
                            nc.vector.tensor_add(x1s32[:, sl], xs32_sb[:], ps[:])
            xtp_stack.close()

            # =================== Phase B: qkv + rope + phi + gate logits =
            bc_stack = contextlib.ExitStack()
            bc = bc_stack.enter_context(tc.tile_pool(name="bc", bufs=1))
            qphi = bc.tile([P, T], bf16, tag="qphi")
            kphi = bc.tile([P, T], bf16, tag="kphi")
            vbf = bc.tile([P, T], bf16, tag="vbf")
            with tc.tile_pool(name="pb", bufs=2) as pb, \
                 tc.tile_pool(name="pb_ps", bufs=3, space="PSUM") as pb_ps:
                for t in range(NT):
                    sl = slice(t * TC, (t + 1) * TC)
                    for which, dst in ((0, qphi), (1, kphi)):
                        ps = pb_ps.tile([P, TC], f32, tag="mm")
                        for k in range(8):
                            nc.tensor.matmul(
                                ps[:],
                                lhsT=wqkv_sb[:, k * 384 + which * P:
                                             k * 384 + which * P + P],
                                rhs=x1f[k][:, sl],
                                start=(k == 0), stop=(k == 7))
                        qc = pb.tile([P, TC], bf16, tag="qc")
                        nc.scalar.activation(qc[:], ps[:], AF.Copy)
                        ps2 = pb_ps.tile([P, TC], f32, tag="mm")
                        nc.tensor.matmul(ps2[:], lhsT=p64_sb[:], rhs=qc[:],
                                         start=True, stop=True)
                        qsw = pb.tile([P, TC], bf16, tag="qsw")
                        nc.scalar.activation(qsw[:], ps2[:], AF.Copy)
                        t1 = pb.tile([P, TC], bf16, tag="t1")
                        nc.vector.tensor_mul(t1[:], qc[:], ctab_sb[:, sl])
                        t2 = pb.tile([P, TC], bf16, tag="t2")
                        nc.vector.tensor_mul(t2[:], qsw[:], stab_sb[:, sl])
                        qr = pb.tile([P, TC], bf16, tag="qr")
                        nc.vector.tensor_add(qr[:], t1[:], t2[:])
                        # phi = min(exp(qr),1) + relu(qr)
                        ex = pb.tile([P, TC], bf16, tag="ex")
                        nc.scalar.activation(ex[:], qr[:], AF.Exp)
                        rl = pb.tile([P, TC], bf16, tag="rl")
                        nc.vector.tensor_scalar_max(rl[:], qr[:], 0.0)
                        nc.vector.scalar_tensor_tensor(
                            out=dst[:, sl], in0=ex[:], scalar=1.0, in1=rl[:],
                            op0=OP.min, op1=OP.add)
                    # v projection
                    ps = pb_ps.tile([P, TC], f32, tag="mm")
                    for k in range(8):
                        nc.tensor.matmul(
                            ps[:],
                            lhsT=wqkv_sb[:, k * 384 + 2 * P: k * 384 + 3 * P],
                            rhs=x1f[k][:, sl],
                            start=(k == 0), stop=(k == 7))
                    nc.scalar.activation(vbf[:, sl], ps[:], AF.Copy)
                    # gate logits G = W_g @ x1 (x1f chunk freed for attnf after)
                    ps = pb_ps.tile([P, TC], f32, tag="mm")
                    for k in range(8):
                        nc.tensor.matmul(ps[:], lhsT=wg_sb[:, k * P:(k + 1) * P],
                                         rhs=x1f[k][:, sl],
                                         start=(k == 0), stop=(k == 7))
                    nc.scalar.activation(g_sb[:, 1 + t * TC: 1 + (t + 1) * TC],
                                         ps[:], AF.Copy)
                    if t == 0:
                        nc.scalar.activation(g_sb[:, 0:1], ps[:, 0:1], AF.Copy)

            # =================== Phase C: chunked linear attention ========
            attn_sh = bc.tile([P, T], bf16, tag="attn_sh")
            attnf = x1f  # gathered attention reuses x1f (per-chunk columns)
            with tc.tile_pool(name="pc", bufs=2) as pc, \
                 tc.tile_pool(name="pc_ps", bufs=2, space="PSUM") as pc_ps, \
                 tc.tile_pool(name="pc_kv", bufs=1, space="PSUM") as pc_kv, \
                 tc.tile_pool(name="pc_num", bufs=2, space="PSUM") as pc_num:
                kv_ps = pc_kv.tile([P, 65], f32, tag="kv")
                nc.vector.memset(kv_ps[:], 0.0)
                for ci in range(NCH):
                    sl = slice(ci * CH, (ci + 1) * CH)
                    # token-major copies of k, v
                    tp1 = pc_ps.tile([P, P], bf16, tag="tp")
                    nc.tensor.transpose(tp1[:], kphi[:, sl], ident_sb[:])
                    ktok = pc.tile([P, P], bf16, tag="ktok")
                    nc.scalar.activation(ktok[:], tp1[:], AF.Copy)
                    tp2 = pc_ps.tile([P, P], bf16, tag="tp")
                    nc.tensor.transpose(tp2[:], vbf[:, sl], ident_sb[:])
                    vtok = pc.tile([P, P], bf16, tag="vtok")
                    nc.scalar.activation(vtok[:], tp2[:], AF.Copy)
                    if ci > 0:
                        kv_sb = pc.tile([P, 65], bf16, tag="kvsb")
                        nc.scalar.activation(kv_sb[:], kv_ps[:], AF.Copy)
                    atn = pc.tile([P, P], bf16, tag="atn")
                    nm = pc_num.tile([P, 130], f32, tag="num")
                    for h in (0, 1):
                        hs = slice(64 * h, 64 * h + 64)
                        ns = slice(65 * h, 65 * h + 64)
                        st_ps = pc_ps.tile([P, P], f32, tag="mm")
                        nc.tensor.matmul(st_ps[:], lhsT=kphi[hs, sl],
                                         rhs=qphi[hs, sl], start=True, stop=True)
                        stm = pc.tile([P, P], bf16, tag="stm")
                        nc.vector.tensor_mul(stm[:], st_ps[:], mask_sb[:])
                        nc.tensor.matmul(nm[:, ns], lhsT=stm[:],
                                         rhs=vtok[:, hs], start=(h == 0),
                                         stop=(ci == 0), skip_group_check=True)
                        nc.tensor.matmul(nm[:, 65 * h + 64:65 * h + 65], lhsT=stm[:],
                                         rhs=ones128[:], start=False,
                                         stop=(ci == 0), skip_group_check=True)
                        if ci > 0:
                            nc.tensor.matmul(nm[:, ns], lhsT=qphi[hs, sl],
                                             rhs=kv_sb[hs, 0:64], start=False,
                                             stop=True, skip_group_check=True)
                            nc.tensor.matmul(nm[:, 65 * h + 64:65 * h + 65],
                                             lhsT=qphi[hs, sl],
                                             rhs=kv_sb[hs, 64:65], start=False,
                                             stop=True, skip_group_check=True)
                    den2 = pc.tile([P, 2], f32, tag="den2")
                    den_ap = bass.AP(tensor=nm.tensor, offset=nm.offset + 64,
                                     ap=[list(nm.ap[0]), [65, 2]])
                    nc.vector.tensor_scalar_add(den2[:], den_ap, 1e-6)
                    nc.vector.reciprocal(den2[:], den2[:])
                    for h in (0, 1):
                        nc.vector.tensor_scalar_mul(
                            atn[:, 64 * h:64 * h + 64], nm[:, 65 * h:65 * h + 64],
                            den2[:, h:h + 1])
                    # state update (after kv_sb snapshot)
                    nc.tensor.matmul(kv_ps[0:64, 0:64], lhsT=ktok[:, 0:64],
                                     rhs=vtok[:, 0:64], start=False,
                                     stop=(ci == NCH - 1), skip_group_check=True)
                    nc.tensor.matmul(kv_ps[64:128, 0:64], lhsT=ktok[:, 64:128],
                                     rhs=vtok[:, 64:128], start=False,
                                     stop=(ci == NCH - 1), tile_position=(0, 64),
                                     skip_group_check=True)
                    nc.tensor.matmul(kv_ps[:, 64:65], lhsT=ktok[:],
                                     rhs=ones128[:], start=False,
                                     stop=(ci == NCH - 1), skip_group_check=True)
                    tp3 = pc_ps.tile([P, P], bf16, tag="tp")
                    nc.tensor.transpose(tp3[:], atn[:], ident_sb[:])
                    nc.scalar.activation(attn_sh[:, sl], tp3[:], AF.Copy)
                    if ci % 4 == 3:
                        t = ci // 4
                        tsl = slice(t * TC, (t + 1) * TC)
                        nc.scalar.dma_start(out=ag2_in[t][:],
                                            in_=attn_sh[:, tsl])
                        nc.gpsimd.collective_compute(
                            "AllGather", mybir.AluOpType.bypass,
                            replica_groups=rg,
                            ins=[ag2_in[t][:]], outs=[ag2_out[t][:]])
                        for i in range(8):
                            nc.sync.dma_start(
                                out=attnf[i][:, tsl],
                                in_=ag2_out[t][i * P:(i + 1) * P, :])
            bc_stack.close()

            # =================== Phase D: w_o + gate + x2 + LN stats ======
            x2bf = late.tile([P, T], bf16, tag="x2bf")
            with tc.tile_pool(name="pd", bufs=2) as pd, \
                 tc.tile_pool(name="pd_ps", bufs=2, space="PSUM") as pd_ps, \
                 tc.tile_pool(name="pd_st", bufs=2, space="PSUM") as pd_st:
                for t in range(NT):
                    sl = slice(t * TC, (t + 1) * TC)
                    ps = pd_ps.tile([P, TC], f32, tag="mm")
                    for k in range(8):
                        nc.tensor.matmul(ps[:], lhsT=wo_sb[:, k * P:(k + 1) * P],
                                         rhs=attnf[k][:, sl],
                                         start=(k == 0), stop=(k == 7))
                    gl = pd.tile([P, TC], bf16, tag="gl")
                    nc.vector.tensor_sub(gl[:], g_sb[:, 1 + t * TC: 1 + (t + 1) * TC],
                                         g_sb[:, t * TC:(t + 1) * TC])
                    gate = pd.tile([P, TC], f32, tag="gate")
                    nc.scalar.activation(gate[:], gl[:], AF.Sigmoid, bias=bg_sb[:])
                    ga = pd.tile([P, TC], f32, tag="ga")
                    nc.vector.tensor_mul(ga[:], gate[:], ps[:])
                    nc.vector.tensor_add(x2s32[:, sl], x1s32[:, sl], ga[:])
                    # LN stats for this chunk
                    nc.scalar.activation(x2bf[:, sl], x2s32[:, sl], AF.Copy)
                    x2sq = pd.tile([P, TC], bf16, tag="x2sq")
                    nc.scalar.activation(x2sq[:], x2bf[:, sl], AF.Square)
                    sp1 = pd_st.tile([1, TC], f32, tag="stat1")
                    nc.tensor.matmul(sp1[:], lhsT=ones128[:], rhs=x2bf[:, sl],
                                     start=True, stop=True)
                    sp2 = pd_st.tile([1, TC], f32, tag="stat2")
                    nc.tensor.matmul(sp2[:], lhsT=ones128[:], rhs=x2sq[:],
                                     start=True, stop=True)
                    st1 = pd.tile([1, TC], f32, tag="st1")
                    nc.scalar.activation(st1[:], sp1[:], AF.Copy)
                    st2 = pd.tile([1, TC], f32, tag="st2")
                    nc.scalar.activation(st2[:], sp2[:], AF.Copy)
                    nc.scalar.dma_start(out=ar_in[t][0:1, :], in_=st1[:])
                    nc.scalar.dma_start(out=ar_in[t][1:2, :], in_=st2[:])
                    nc.gpsimd.collective_compute(
                        "AllReduce", mybir.AluOpType.add, replica_groups=rg,
                        ins=[ar_in[t][:]], outs=[ar_out[t][:]])
            wx1_stack.close()

            # =================== Phase E: LayerNorm apply =================
            h_sh = late.tile([P, T], bf16, tag="h_sh")
            hf = attnf  # gathered h reuses x1f tiles (per-chunk columns)
            with tc.tile_pool(name="pe", bufs=2) as pe, \
                 tc.tile_pool(name="pe_ps", bufs=2, space="PSUM") as pe_ps:
                for t in range(NT):
                    sl = slice(t * TC, (t + 1) * TC)
                    s1row = pe.tile([1, TC], f32, tag="s1row")
                    nc.sync.dma_start(out=s1row[:], in_=ar_out[t][0:1, :])
                    s2row = pe.tile([1, TC], f32, tag="s2row")
                    nc.sync.dma_start(out=s2row[:], in_=ar_out[t][1:2, :])
                    s1b = pe_ps.tile([P, TC], f32, tag="s1b")
                    nc.tensor.matmul(s1b[:], lhsT=ones1f[:], rhs=s1row[:],
                                     start=True, stop=True)
                    s2b = pe_ps.tile([P, TC], f32, tag="s2b")
                    nc.tensor.matmul(s2b[:], lhsT=ones1f[:], rhs=s2row[:],
                                     start=True, stop=True)
                    # hp = x2 - mu ; mu2 = (s1b/D)^2 ; var = s2b/D - mu2
                    hp = pe.tile([P, TC], f32, tag="hp")
                    nc.vector.scalar_tensor_tensor(
                        out=hp[:], in0=s1b[:], scalar=-1.0 / D,
                        in1=x2s32[:, sl], op0=OP.mult, op1=OP.add)
                    mu2 = pe.tile([P, TC], f32, tag="mu2")
                    nc.scalar.activation(mu2[:], s1b[:], AF.Square, scale=1.0 / D)
                    var = pe.tile([P, TC], f32, tag="var")
                    nc.vector.scalar_tensor_tensor(
                        out=var[:], in0=s2b[:], scalar=1.0 / D,
                        in1=mu2[:], op0=OP.mult, op1=OP.subtract)
                    sd = pe.tile([P, TC], f32, tag="sd")
                    nc.scalar.activation(sd[:], var[:], AF.Sqrt, bias=eps128[:])
                    rstd = pe.tile([P, TC], f32, tag="rstd")
                    nc.vector.reciprocal(rstd[:], sd[:])
                    h2 = pe.tile([P, TC], f32, tag="h2")
                    nc.vector.tensor_mul(h2[:], hp[:], rstd[:])
                    nc.vector.tensor_scalar(
                        out=h_sh[:, sl], in0=h2[:], scalar1=lng_sb[:],
                        scalar2=lnb_sb[:], op0=OP.mult, op1=OP.add)
                    nc.scalar.dma_start(out=ag3_in[t][:], in_=h_sh[:, sl])
                    nc.gpsimd.collective_compute(
                        "AllGather", mybir.AluOpType.bypass, replica_groups=rg,
                        ins=[ag3_in[t][:]], outs=[ag3_out[t][:]])
                    for i in range(8):
                        nc.sync.dma_start(out=hf[i][:, sl],
                                          in_=ag3_out[t][i * P:(i + 1) * P, :])

            # =================== Phase F: MoE =============================
            with tc.tile_pool(name="pf", bufs=2) as pf, \
                 tc.tile_pool(name="pf_hid", bufs=2) as pf_hid, \
                 tc.tile_pool(name="pf_ps", bufs=2, space="PSUM") as pf_ps, \
                 tc.tile_pool(name="pf_ps2", bufs=2, space="PSUM") as pf_ps2, \
                 tc.tile_pool(name="pf_gw", bufs=1, space="PSUM") as pf_gw:
                for t in range(NT):
                    sl = slice(t * TC, (t + 1) * TC)
                    # gate weight row for this core's expert
                    lg = pf_gw.tile([4, TC], f32, tag="lg")
                    for k in range(8):
                        nc.tensor.matmul(lg[:], lhsT=wmg_sb[:, k * 4:(k + 1) * 4],
                                         rhs=hf[k][:, sl],
                                         start=(k == 0), stop=(k == 7))
                    gx = pf.tile([4, TC], bf16, tag="gx")
                    nc.scalar.activation(gx[:], lg[:], AF.Exp)
                    sm = pf_gw.tile([1, TC], f32, tag="sm")
                    nc.tensor.matmul(sm[:], lhsT=ones4[:], rhs=gx[:],
                                     start=True, stop=True)
                    sel = pf_gw.tile([1, TC], f32, tag="sel")
                    nc.tensor.matmul(sel[:], lhsT=esel_sb[:], rhs=gx[:],
                                     start=True, stop=True)
                    rc = pf.tile([1, TC], f32, tag="rc")
                    nc.vector.reciprocal(rc[:], sm[:])
                    gwrow = pf.tile([1, TC], f32, tag="gwrow")
                    nc.vector.tensor_mul(gwrow[:], sel[:], rc[:])
                    gwb_ps = pf_gw.tile([P, TC], f32, tag="gwb_ps")
                    nc.tensor.matmul(gwb_ps[:], lhsT=ones1f[:], rhs=gwrow[:],
                                     start=True, stop=True)
                    gwb = pf.tile([P, TC], f32, tag="gwb")
                    nc.scalar.activation(gwb[:], gwb_ps[:], AF.Copy)

                    # hid = silu(w1 @ h + b1)
                    hid = [pf_hid.tile([P, TC], bf16, tag=f"hid{ft}",
                                       name=f"hid{ft}") for ft in range(16)]
                    for ft in range(16):
                        hp = pf_ps.tile([P, TC], f32, tag="hid")
                        for k in range(8):
                            nc.tensor.matmul(
                                hp[:],
                                lhsT=w1_sb[k][:, ft * P:(ft + 1) * P],
                                rhs=hf[k][:, sl], start=(k == 0), stop=(k == 7))
                        sg = pf.tile([P, TC], bf16, tag="sg")
                        nc.scalar.activation(sg[:], hp[:], AF.Sigmoid,
                                             bias=b1s_sb[:, ft:ft + 1])
                        nc.vector.scalar_tensor_tensor(
                            out=hid[ft][:], in0=hp[:], scalar=b1s_sb[:, ft:ft + 1],
                            in1=sg[:], op0=OP.add, op1=OP.mult)
                    for dt in range(8):
                        op = pf_ps2.tile([P, TC], f32, tag="out")
                        for ft in range(16):
                            nc.tensor.matmul(
                                op[:],
                                lhsT=w2_sb[ft][:, dt * P:(dt + 1) * P],
                                rhs=hid[ft][:], start=(ft == 0), stop=False,
                                skip_group_check=True)
                        nc.tensor.matmul(op[:], lhsT=b2s_sb[:, dt * P:(dt + 1) * P],
                                         rhs=ones512[:], start=False, stop=True,
                                         skip_group_check=True)
                        par = pf.tile([P, TC], bf16, tag="par")
                        nc.vector.tensor_mul(par[:], op[:], gwb[:])
                        nc.scalar.dma_start(out=rs_in[t][dt * P:(dt + 1) * P, :],
                                            in_=par[:])
                    nc.gpsimd.collective_compute(
                        "ReduceScatter", mybir.AluOpType.add, replica_groups=rg,
                        ins=[rs_in[t][:]], outs=[rs_out[t][:]])
                    fo = pf.tile([P, TC], bf16, tag="fo")
                    nc.sync.dma_start(out=fo[:], in_=rs_out[t][:])
                    yo = pf.tile([P, TC], f32, tag="yo")
                    nc.vector.tensor_add(yo[:], x2s32[:, sl], fo[:])
                    nc.scalar.dma_start(out=y_d[:, sl], in_=yo[:])
            late_stack.close()
    nc.finalize()
    return nc


def _prep_inputs(inputs):
    x = np.asarray(inputs["x"])[0]          # [T, D] f32
    w_dw = np.asarray(inputs["w_dw"])
    w_pw = np.asarray(inputs["w_pw"])
    w_qkv = np.asarray(inputs["w_qkv"])
    w_o = np.asarray(inputs["w_o"])
    w_gate = np.asarray(inputs["w_gate"])
    b_gate = np.asarray(inputs["b_gate"])
    ln_g = np.asarray(inputs["ln_g"])
    ln_b = np.asarray(inputs["ln_b"])
    w_mg = np.asarray(inputs["w_moe_gate"])
    w1 = np.asarray(inputs["w1"])
    b1 = np.asarray(inputs["b1"])
    w2 = np.asarray(inputs["w2"])
    b2 = np.asarray(inputs["b2"])

    xT = np.ascontiguousarray(x.T)                       # [D, T]
    xtp = np.zeros((D, T + 2), dtype=BF)
    xtp[:, 2:] = xT.astype(BF)

    # depthwise taps, rotated per-core below: wdw[p, 3k+j] = w_dw[rot_k*128+p, j]
    wdw_all = w_dw.astype(np.float32)                    # [1024, 3]

    # rope tables
    inv_freq = 1.0 / (10000.0 ** (np.arange(0, DH, 2, dtype=np.float32) / DH))
    pos = np.arange(T, dtype=np.float32)
    theta = pos[None, :] * inv_freq[:, None]             # [32, T]
    cos64 = np.concatenate([np.cos(theta), np.cos(theta)], axis=0)
    sin64 = np.concatenate([-np.sin(theta), np.sin(theta)], axis=0)
    ctab = np.tile(cos64, (2, 1)).astype(BF)             # [128, T]
    stab = np.tile(sin64, (2, 1)).astype(BF)

    p64 = np.zeros((P, P), dtype=BF)
    for r in range(P):
        p64[r, (r % 64 + 32) % 64 + 64 * (r // 64)] = 1.0
    # p64 is used as lhsT: out[i,t] = sum_k p64[k,i] q[k,t] = q[swap(i),t]

    mask = np.triu(np.ones((P, P), np.float32))          # [s, t] keep s<=t

    perm = np.concatenate([np.arange(0, DH, 2), np.arange(1, DH, 2)])
    wq, wk, wv = w_qkv[0:D], w_qkv[D:2 * D], w_qkv[2 * D:3 * D]

    in_maps = []
    for c in range(NC_N):
        heads = [2 * c, 2 * c + 1]
        qrows = np.concatenate([h * DH + perm for h in heads])
        krows = qrows
        vrows = np.concatenate([np.arange(h * DH, (h + 1) * DH) for h in heads])
        wqkvT = np.concatenate(
            [wq[qrows].T, wk[krows].T, wv[vrows].T], axis=1)   # [1024, 384]
        e, hh = c // 2, c % 2
        w1s = w1[e, hh * 2048:(hh + 1) * 2048, :].T          # [1024, 2048]
        b1s = b1[e, hh * 2048:(hh + 1) * 2048]               # [2048]
        w2s = w2[e, :, hh * 2048:(hh + 1) * 2048].T          # [2048, 1024]
        b2s = (b2[e] if hh == 0 else np.zeros(D, np.float32))
        esel = np.zeros((4, 1), dtype=BF)
        esel[e, 0] = 1.0
        # conv pointwise, all 8 out-blocks; both row- (contraction) and
        # col- (output) blocks rotated so block 0 is this core's own rows.
        # xtp and wdw below use the same row rotation.
        wpwT = w_pw.T.astype(BF)                              # [in, out]
        wpwF = np.empty((P, 64 * P), dtype=BF)
        for k in range(8):
            for i in range(8):
                ok, oi = (c + k) % 8, (c + i) % 8
                wpwF[:, (k * 8 + i) * P:(k * 8 + i + 1) * P] = \
                    wpwT[ok * P:(ok + 1) * P, oi * P:(oi + 1) * P]
        rows = np.concatenate(
            [np.arange(((c + j) % 8) * P, ((c + j) % 8) * P + P)
             for j in range(8)])
        xtp_c = np.ascontiguousarray(xtp[rows])
        wdw_c = np.ascontiguousarray(wdw_all[rows])           # [1024, 3] rot
        wdw_c = wdw_c.reshape(8, P, 3).transpose(1, 0, 2).reshape(P, 24)
        w2s_packed = np.empty((P, 16 * 1024), dtype=BF)
        for k in range(16):
            w2s_packed[:, k * 1024:(k + 1) * 1024] = \
                w2s[k * P:(k + 1) * P].astype(BF)
        b1sp = b1s.reshape(16, P).T.astype(np.float32)
        in_maps.append({
            "xtp": xtp_c,
            "xs32": np.ascontiguousarray(xT[c * P:(c + 1) * P]).astype(np.float32),
            "wdw": np.ascontiguousarray(wdw_c).astype(np.float32),
            "wpw": wpwF,
            "wqkv": _pack_k(wqkvT.astype(BF), 384, rot=c),
            "ctab": ctab, "stab": stab, "p64": p64, "mask": mask,
            "wo": _pack_k(w_o.T[:, c * P:(c + 1) * P].astype(BF), P),
            "wg": _pack_k(w_gate.T[:, c * P:(c + 1) * P].astype(BF), P, rot=c),
            "bg": b_gate[c * P:(c + 1) * P].reshape(P, 1).astype(np.float32),
            "lng": ln_g[c * P:(c + 1) * P].reshape(P, 1).astype(np.float32),
            "lnb": ln_b[c * P:(c + 1) * P].reshape(P, 1).astype(np.float32),
            "wmg": _pack_k(w_mg.T.astype(BF), 4),
            "esel": esel,
            "w1s": _pack_k(w1s.astype(BF), 2048),
            "b1s": b1sp,
            "w2s": w2s_packed,
            "b2s": b2s.reshape(1, D).astype(BF),
        })
    return in_maps


def _pack_k(mat_km, M, rot=0):
    """[1024, M] -> [128, 8*M]: k-block i holds rows of block (rot+i)%8.

    rot matches the conv out-block rotation: x1f[i] holds x1 rows
    ((rot+i)%8)*128.., so contraction block i must use those weight rows.
    """
    out = np.empty((P, 8 * M), dtype=mat_km.dtype)
    for k in range(8):
        o = (rot + k) % 8
        out[:, k * M:(k + 1) * M] = mat_km[o * P:(o + 1) * P]
    return out


def kernel(**inputs) -> np.ndarray:
    global _PROGRAM
    from concourse.bass_utils import run_bass_kernel_spmd

    if _PROGRAM is None:
        _PROGRAM = _build_program()
    nc = _PROGRAM
    in_maps = _prep_inputs(inputs)
    last_err = None
    for _attempt in range(2):
        try:
            res = run_bass_kernel_spmd(nc, in_maps, list(range(NC_N)))
            break
        except Exception as exc:  # transient device hiccups: retry once
            last_err = exc
    else:
        raise last_err
    outT = np.empty((D, T), dtype=np.float32)
    for c in range(NC_N):
        outT[c * P:(c + 1) * P] = res.results[c]["y"]
    return np.ascontiguousarray(outT.T)[None, :, :].astype(np.float32)


# revision 46
# speedup vs baseline: 1.1062x; 1.1062x over previous
"""Trainium2 Bass kernel for nn_DeltaNet_19430432047178.

Strategy (8 cores, SPMD):
  - activations live transposed on device: [d, T] with d on partitions
  - depthwise conv k=3 computed on the vector engine (3 shifted
    per-partition FMAs); pointwise conv is a single row-sharded matmul
  - attention tensor-parallel over heads (2 heads/core), chunked linear
    attention (C=128) with KV state accumulated in PSUM
  - w_o / delta-gate / LN row-sharded over d
  - MoE sharded expert x hidden-half (core c -> expert c//2, half c%2)
  - all collectives are chunked over T (4 chunks of 512) and pipelined
    under compute: 4x AG(x1), 4x AG(attn), 4x AR(LN stats), 4x AG(h),
    4x RS(ffn partial, bf16)
  - gpsimd issues only collectives; data DMAs ride sync/scalar (HWDGE)
  - all matmuls bf16 with f32 PSUM accumulation
"""
import numpy as np
import ml_dtypes


NC_N = 8
T = 2048
D = 1024
H = 16
DH = 64
E = 4
HD = 4096
P = 128
CH = 128            # attention chunk
NCH = T // CH       # 16
NT = 4              # T chunks of 512 for GEMMs and collectives
TC = 512

BF = ml_dtypes.bfloat16

_PROGRAM = None  # cached nc


def _build_program():
    import concourse.mybir as mybir
    import concourse.tile as tile
    from concourse import bacc
    from concourse.masks import make_identity

    f32 = mybir.dt.float32
    bf16 = mybir.dt.bfloat16
    AF = mybir.ActivationFunctionType
    OP = mybir.AluOpType

    nc = bacc.Bacc()

    # ---------------- external params (per-core) ----------------
    xtp_d = nc.declare_dram_parameter("xtp", [D, T + 2], bf16, isOutput=False)
    xs32_d = nc.declare_dram_parameter("xs32", [P, T], f32, isOutput=False)
    wdw_d = nc.declare_dram_parameter("wdw", [P, 24], f32, isOutput=False)
    wpw_d = nc.declare_dram_parameter("wpw", [P, 8 * P], bf16, isOutput=False)
    wqkv_d = nc.declare_dram_parameter("wqkv", [P, 8 * 384], bf16, isOutput=False)
    ctab_d = nc.declare_dram_parameter("ctab", [P, T], bf16, isOutput=False)
    stab_d = nc.declare_dram_parameter("stab", [P, T], bf16, isOutput=False)
    p64_d = nc.declare_dram_parameter("p64", [P, P], bf16, isOutput=False)
    mask_d = nc.declare_dram_parameter("mask", [P, P], f32, isOutput=False)
    wo_d = nc.declare_dram_parameter("wo", [P, 8 * P], bf16, isOutput=False)
    wg_d = nc.declare_dram_parameter("wg", [P, 8 * P], bf16, isOutput=False)
    bg_d = nc.declare_dram_parameter("bg", [P, 1], f32, isOutput=False)
    lng_d = nc.declare_dram_parameter("lng", [P, 1], f32, isOutput=False)
    lnb_d = nc.declare_dram_parameter("lnb", [P, 1], f32, isOutput=False)
    wmg_d = nc.declare_dram_parameter("wmg", [P, 8 * 4], bf16, isOutput=False)
    esel_d = nc.declare_dram_parameter("esel", [4, 1], bf16, isOutput=False)
    w1s_d = nc.declare_dram_parameter("w1s", [P, 8 * 2048], bf16, isOutput=False)
    b1s_d = nc.declare_dram_parameter("b1s", [P, 16], f32, isOutput=False)
    w2s_d = nc.declare_dram_parameter("w2s", [P, 16 * 1024], bf16, isOutput=False)
    b2s_d = nc.declare_dram_parameter("b2s", [1, 1024], bf16, isOutput=False)
    y_d = nc.declare_dram_parameter("y", [P, T], f32, isOutput=True)

    # ---------------- internal DRAM (chunked collectives) ----------------
    rg = [list(range(NC_N))]
    dum_in = nc.dram_tensor("dum_in", [P, 16], bf16)
    dum_out = nc.dram_tensor("dum_out", [D, 16], bf16, addr_space="Shared")
    ag1_in = [nc.dram_tensor(f"ag1_in{t}", [P, TC], bf16) for t in range(NT)]
    ag1_out = [nc.dram_tensor(f"ag1_out{t}", [D, TC], bf16, addr_space="Shared")
               for t in range(NT)]
    ag2_in = [nc.dram_tensor(f"ag2_in{t}", [P, TC], bf16) for t in range(NT)]
    ag2_out = [nc.dram_tensor(f"ag2_out{t}", [D, TC], bf16, addr_space="Shared")
               for t in range(NT)]
    ar_in = [nc.dram_tensor(f"ar_in{t}", [2, TC], f32) for t in range(NT)]
    ar_out = [nc.dram_tensor(f"ar_out{t}", [2, TC], f32, addr_space="Shared")
              for t in range(NT)]
    ag3_in = [nc.dram_tensor(f"ag3_in{t}", [P, TC], bf16) for t in range(NT)]
    ag3_out = [nc.dram_tensor(f"ag3_out{t}", [D, TC], bf16, addr_space="Shared")
               for t in range(NT)]
    rs_in = [nc.dram_tensor(f"rs_in{t}", [D, TC], bf16) for t in range(NT)]
    rs_out = [nc.dram_tensor(f"rs_out{t}", [P, TC], bf16) for t in range(NT)]

    import concourse.bass as bass
    import contextlib

    with tile.TileContext(nc) as tc:
        with contextlib.ExitStack() as stack:
            consts = stack.enter_context(tc.tile_pool(name="consts", bufs=1))

            # ---- constants / weights ----
            wdw_sb = consts.tile([P, 24], f32, tag="wdw")
            nc.scalar.dma_start(out=wdw_sb[:], in_=wdw_d[:])
            wpw_sb = consts.tile([P, 8 * P], bf16, tag="wpw")
            nc.scalar.dma_start(out=wpw_sb[:], in_=wpw_d[:])
            wqkv_sb = consts.tile([P, 8 * 384], bf16, tag="wqkv")
            nc.scalar.dma_start(out=wqkv_sb[:], in_=wqkv_d[:])
            wo_sb = consts.tile([P, 8 * P], bf16, tag="wo")
            nc.scalar.dma_start(out=wo_sb[:], in_=wo_d[:])
            wg_sb = consts.tile([P, 8 * P], bf16, tag="wg")
            nc.scalar.dma_start(out=wg_sb[:], in_=wg_d[:])
            bg_sb = consts.tile([P, 1], f32, tag="bg")
            nc.scalar.dma_start(out=bg_sb[:], in_=bg_d[:])
            lng_sb = consts.tile([P, 1], f32, tag="lng")
            nc.scalar.dma_start(out=lng_sb[:], in_=lng_d[:])
            lnb_sb = consts.tile([P, 1], f32, tag="lnb")
            nc.scalar.dma_start(out=lnb_sb[:], in_=lnb_d[:])
            wmg_sb = consts.tile([P, 8 * 4], bf16, tag="wmg")
            nc.scalar.dma_start(out=wmg_sb[:], in_=wmg_d[:])
            esel_sb = consts.tile([4, 1], bf16, tag="esel")
            nc.scalar.dma_start(out=esel_sb[:], in_=esel_d[:])
            b1s_sb = consts.tile([P, 16], f32, tag="b1s")
            nc.scalar.dma_start(out=b1s_sb[:], in_=b1s_d[:])
            b2s_sb = consts.tile([1, 1024], bf16, tag="b2s")
            nc.scalar.dma_start(out=b2s_sb[:], in_=b2s_d[:])
            ctab_sb = consts.tile([P, T], bf16, tag="ctab")
            nc.scalar.dma_start(out=ctab_sb[:], in_=ctab_d[:])
            stab_sb = consts.tile([P, T], bf16, tag="stab")
            nc.scalar.dma_start(out=stab_sb[:], in_=stab_d[:])
            p64_sb = consts.tile([P, P], bf16, tag="p64")
            nc.scalar.dma_start(out=p64_sb[:], in_=p64_d[:])
            mask_sb = consts.tile([P, P], f32, tag="mask")
            nc.scalar.dma_start(out=mask_sb[:], in_=mask_d[:])
            ident_sb = consts.tile([P, P], bf16, tag="ident")
            make_identity(nc, ident_sb[:])
            ones128 = consts.tile([P, 1], bf16, tag="ones128")
            nc.vector.memset(ones128[:], 1.0)
            ones4 = consts.tile([4, 1], bf16, tag="ones4")
            nc.vector.memset(ones4[:], 1.0)
            ones512 = consts.tile([1, TC], bf16, tag="ones512")
            nc.vector.memset(ones512[:], 1.0)
            ones1f = consts.tile([1, P], f32, tag="ones1f")
            nc.vector.memset(ones1f[:], 1.0)
            eps128 = consts.tile([P, 1], f32, tag="eps128")
            nc.vector.memset(eps128[:], 1e-5)

            # warm up collectives (barrier + first-RDH cost) while DMAs run
            dum_sb = consts.tile([P, 16], bf16, tag="dum")
            nc.vector.memset(dum_sb[:], 0.0)
            nc.scalar.dma_start(out=dum_in[:], in_=dum_sb[:])
            nc.gpsimd.collective_compute(
                "AllGather", mybir.AluOpType.bypass, replica_groups=rg,
                ins=[dum_in[:]], outs=[dum_out[:]])

            # MoE weights prefetch (sync queue; needed from ~mid-kernel)
            w1_sb = [consts.tile([P, 2048], bf16, tag=f"w1_{k}", name=f"w1_{k}")
                     for k in range(8)]
            for k in range(8):
                nc.sync.dma_start(out=w1_sb[k][:],
                                  in_=w1s_d[:, k * 2048:(k + 1) * 2048])
            w2_sb = [consts.tile([P, 1024], bf16, tag=f"w2_{k}", name=f"w2_{k}")
                     for k in range(16)]
            for k in range(16):
                nc.sync.dma_start(out=w2_sb[k][:],
                                  in_=w2s_d[:, k * 1024:(k + 1) * 1024])

            # persistent activations
            late_stack = contextlib.ExitStack()
            late = late_stack.enter_context(tc.tile_pool(name="late", bufs=1))
            x2s32 = late.tile([P, T], f32, tag="x2s32")
            x1f = [late.tile([P, T], bf16, tag=f"x1f{i}", name=f"x1f{i}")
                   for i in range(8)]
            wx1_stack = contextlib.ExitStack()
            wx1 = wx1_stack.enter_context(tc.tile_pool(name="wx1", bufs=1))
            x1s32 = wx1.tile([P, T], f32, tag="x1s32")
            g_sb = wx1.tile([P, T + 1], bf16, tag="g_sb")

            # =================== Phase A: depthwise (DVE) + pointwise ====
            xtp_stack = contextlib.ExitStack()
            xtp_pool = xtp_stack.enter_context(tc.tile_pool(name="xtp", bufs=1))
            xtp = [xtp_pool.tile([P, T + 2], bf16, tag=f"xtp{i}", name=f"xtp{i}")
                   for i in range(8)]
            for k in range(8):
                eng = nc.scalar if k % 2 == 0 else nc.sync
                eng.dma_start(out=xtp[k][:], in_=xtp_d[k * P:(k + 1) * P, :])
            with tc.tile_pool(name="pa", bufs=2) as pa, \
                 tc.tile_pool(name="dwy", bufs=2) as dwy_pool, \
                 tc.tile_pool(name="pa_ps", bufs=3, space="PSUM") as pa_ps:
                for t in range(NT):
                    sl = slice(t * TC, (t + 1) * TC)
                    xs32_sb = pa.tile([P, TC], f32, tag="xs32")
                    nc.scalar.dma_start(out=xs32_sb[:], in_=xs32_d[:, sl])
                    ps = pa_ps.tile([P, TC], f32, tag="mm")
                    for k in range(8):
                        # dwy = sum_j xtp[k][:, t*TC+j : +TC] * wdw[:, 3k+j]
                        d = dwy_pool.tile([P, TC], bf16, tag=f"dwy{k}")
                        nc.vector.tensor_scalar_mul(
                            d[:], xtp[k][:, t * TC:t * TC + TC],
                            wdw_sb[:, 3 * k:3 * k + 1])
                        nc.vector.scalar_tensor_tensor(
                            out=d[:], in0=xtp[k][:, t * TC + 1:t * TC + 1 + TC],
                            scalar=wdw_sb[:, 3 * k + 1:3 * k + 2], in1=d[:],
                            op0=OP.mult, op1=OP.add)
                        nc.vector.scalar_tensor_tensor(
                            out=d[:], in0=xtp[k][:, t * TC + 2:t * TC + 2 + TC],
                            scalar=wdw_sb[:, 3 * k + 2:3 * k + 3], in1=d[:],
                            op0=OP.mult, op1=OP.add)
                        nc.tensor.matmul(
                            ps[:], lhsT=wpw_sb[:, k * P:(k + 1) * P],
                            rhs=d[:],
                            start=(k == 0), stop=(k == 7))
                    nc.vector.tensor_add(x1s32[:, sl], xs32_sb[:], ps[:])
                    x1bf = pa.tile([P, TC], bf16, tag="x1bf")
                    nc.scalar.activation(x1bf[:], x1s32[:, sl], AF.Copy)
                    nc.scalar.dma_start(out=ag1_in[t][:], in_=x1bf[:])
                    nc.gpsimd.collective_compute(
                        "AllGather", mybir.AluOpType.bypass, replica_groups=rg,
                        ins=[ag1_in[t][:]], outs=[ag1_out[t][:]])
                    for i in range(8):
                        nc.sync.dma_start(out=x1f[i][:, sl],
                                          in_=ag1_out[t][i * P:(i + 1) * P, :])
            xtp_stack.close()

            # =================== Phase B: qkv + rope + phi + gate logits =
            bc_stack = contextlib.ExitStack()
            bc = bc_stack.enter_context(tc.tile_pool(name="bc", bufs=1))
            qphi = bc.tile([P, T], bf16, tag="qphi")
            kphi = bc.tile([P, T], bf16, tag="kphi")
            vbf = bc.tile([P, T], bf16, tag="vbf")
            with tc.tile_pool(name="pb", bufs=2) as pb, \
                 tc.tile_pool(name="pb_ps", bufs=3, space="PSUM") as pb_ps:
                for t in range(NT):
                    sl = slice(t * TC, (t + 1) * TC)
                    for which, dst in ((0, qphi), (1, kphi)):
                        ps = pb_ps.tile([P, TC], f32, tag="mm")
                        for k in range(8):
                            nc.tensor.matmul(
                                ps[:],
                                lhsT=wqkv_sb[:, k * 384 + which * P:
                                             k * 384 + which * P + P],
                                rhs=x1f[k][:, sl],
                                start=(k == 0), stop=(k == 7))
                        qc = pb.tile([P, TC], bf16, tag="qc")
                        nc.scalar.activation(qc[:], ps[:], AF.Copy)
                        ps2 = pb_ps.tile([P, TC], f32, tag="mm")
                        nc.tensor.matmul(ps2[:], lhsT=p64_sb[:], rhs=qc[:],
                                         start=True, stop=True)
                        qsw = pb.tile([P, TC], bf16, tag="qsw")
                        nc.scalar.activation(qsw[:], ps2[:], AF.Copy)
                        t1 = pb.tile([P, TC], bf16, tag="t1")
                        nc.vector.tensor_mul(t1[:], qc[:], ctab_sb[:, sl])
                        t2 = pb.tile([P, TC], bf16, tag="t2")
                        nc.vector.tensor_mul(t2[:], qsw[:], stab_sb[:, sl])
                        qr = pb.tile([P, TC], bf16, tag="qr")
                        nc.vector.tensor_add(qr[:], t1[:], t2[:])
                        # phi = min(exp(qr),1) + relu(qr)
                        ex = pb.tile([P, TC], bf16, tag="ex")
                        nc.scalar.activation(ex[:], qr[:], AF.Exp)
                        rl = pb.tile([P, TC], bf16, tag="rl")
                        nc.vector.tensor_scalar_max(rl[:], qr[:], 0.0)
                        nc.vector.scalar_tensor_tensor(
                            out=dst[:, sl], in0=ex[:], scalar=1.0, in1=rl[:],
                            op0=OP.min, op1=OP.add)
                    # v projection
                    ps = pb_ps.tile([P, TC], f32, tag="mm")
                    for k in range(8):
                        nc.tensor.matmul(
                            ps[:],
                            lhsT=wqkv_sb[:, k * 384 + 2 * P: k * 384 + 3 * P],
                            rhs=x1f[k][:, sl],
                            start=(k == 0), stop=(k == 7))
                    nc.scalar.activation(vbf[:, sl], ps[:], AF.Copy)
                    # gate logits G = W_g @ x1 (x1f chunk freed for attnf after)
                    ps = pb_ps.tile([P, TC], f32, tag="mm")
                    for k in range(8):
                        nc.tensor.matmul(ps[:], lhsT=wg_sb[:, k * P:(k + 1) * P],
                                         rhs=x1f[k][:, sl],
                                         start=(k == 0), stop=(k == 7))
                    nc.scalar.activation(g_sb[:, 1 + t * TC: 1 + (t + 1) * TC],
                                         ps[:], AF.Copy)
                    if t == 0:
                        nc.scalar.activation(g_sb[:, 0:1], ps[:, 0:1], AF.Copy)

            # =================== Phase C: chunked linear attention ========
            attn_sh = bc.tile([P, T], bf16, tag="attn_sh")
            attnf = x1f  # gathered attention reuses x1f (per-chunk columns)
            with tc.tile_pool(name="pc", bufs=2) as pc, \
                 tc.tile_pool(name="pc_ps", bufs=2, space="PSUM") as pc_ps, \
                 tc.tile_pool(name="pc_kv", bufs=1, space="PSUM") as pc_kv, \
                 tc.tile_pool(name="pc_num", bufs=2, space="PSUM") as pc_num:
                kv_ps = pc_kv.tile([P, 65], f32, tag="kv")
                nc.vector.memset(kv_ps[:], 0.0)
                for ci in range(NCH):
                    sl = slice(ci * CH, (ci + 1) * CH)
                    # token-major copies of k, v
                    tp1 = pc_ps.tile([P, P], bf16, tag="tp")
                    nc.tensor.transpose(tp1[:], kphi[:, sl], ident_sb[:])
                    ktok = pc.tile([P, P], bf16, tag="ktok")
                    nc.scalar.activation(ktok[:], tp1[:], AF.Copy)
                    tp2 = pc_ps.tile([P, P], bf16, tag="tp")
                    nc.tensor.transpose(tp2[:], vbf[:, sl], ident_sb[:])
                    vtok = pc.tile([P, P], bf16, tag="vtok")
                    nc.scalar.activation(vtok[:], tp2[:], AF.Copy)
                    if ci > 0:
                        kv_sb = pc.tile([P, 65], bf16, tag="kvsb")
                        nc.scalar.activation(kv_sb[:], kv_ps[:], AF.Copy)
                    atn = pc.tile([P, P], bf16, tag="atn")
                    nm = pc_num.tile([P, 130], f32, tag="num")
                    for h in (0, 1):
                        hs = slice(64 * h, 64 * h + 64)
                        ns = slice(65 * h, 65 * h + 64)
                        st_ps = pc_ps.tile([P, P], f32, tag="mm")
                        nc.tensor.matmul(st_ps[:], lhsT=kphi[hs, sl],
                                         rhs=qphi[hs, sl], start=True, stop=True)
                        stm = pc.tile([P, P], bf16, tag="stm")
                        nc.vector.tensor_mul(stm[:], st_ps[:], mask_sb[:])
                        nc.tensor.matmul(nm[:, ns], lhsT=stm[:],
                                         rhs=vtok[:, hs], start=(h == 0),
                                         stop=(ci == 0), skip_group_check=True)
                        nc.tensor.matmul(nm[:, 65 * h + 64:65 * h + 65], lhsT=stm[:],
                                         rhs=ones128[:], start=False,
                                         stop=(ci == 0), skip_group_check=True)
                        if ci > 0:
                            nc.tensor.matmul(nm[:, ns], lhsT=qphi[hs, sl],
                                             rhs=kv_sb[hs, 0:64], start=False,
                                             stop=True, skip_group_check=True)
                            nc.tensor.matmul(nm[:, 65 * h + 64:65 * h + 65],
                                             lhsT=qphi[hs, sl],
                                             rhs=kv_sb[hs, 64:65], start=False,
                                             stop=True, skip_group_check=True)
                    den2 = pc.tile([P, 2], f32, tag="den2")
                    den_ap = bass.AP(tensor=nm.tensor, offset=nm.offset + 64,
                                     ap=[list(nm.ap[0]), [65, 2]])
                    nc.vector.tensor_scalar_add(den2[:], den_ap, 1e-6)
                    nc.vector.reciprocal(den2[:], den2[:])
                    for h in (0, 1):
                        nc.vector.tensor_scalar_mul(
                            atn[:, 64 * h:64 * h + 64], nm[:, 65 * h:65 * h + 64],
                            den2[:, h:h + 1])
                    # state update (after kv_sb snapshot)
                    nc.tensor.matmul(kv_ps[0:64, 0:64], lhsT=ktok[:, 0:64],
                                     rhs=vtok[:, 0:64], start=False,
                                     stop=(ci == NCH - 1), skip_group_check=True)
                    nc.tensor.matmul(kv_ps[64:128, 0:64], lhsT=ktok[:, 64:128],
                                     rhs=vtok[:, 64:128], start=False,
                                     stop=(ci == NCH - 1), tile_position=(0, 64),
                                     skip_group_check=True)
                    nc.tensor.matmul(kv_ps[:, 64:65], lhsT=ktok[:],
                                     rhs=ones128[:], start=False,
                                     stop=(ci == NCH - 1), skip_group_check=True)
                    tp3 = pc_ps.tile([P, P], bf16, tag="tp")
                    nc.tensor.transpose(tp3[:], atn[:], ident_sb[:])
                    nc.scalar.activation(attn_sh[:, sl], tp3[:], AF.Copy)
                    if ci % 4 == 3:
                        t = ci // 4
                        tsl = slice(t * TC, (t + 1) * TC)
                        nc.scalar.dma_start(out=ag2_in[t][:],
                                            in_=attn_sh[:, tsl])
                        nc.gpsimd.collective_compute(
                            "AllGather", mybir.AluOpType.bypass,
                            replica_groups=rg,
                            ins=[ag2_in[t][:]], outs=[ag2_out[t][:]])
                        for i in range(8):
                            nc.sync.dma_start(
                                out=attnf[i][:, tsl],
                                in_=ag2_out[t][i * P:(i + 1) * P, :])
            bc_stack.close()

            # =================== Phase D: w_o + gate + x2 + LN stats ======
            x2bf = late.tile([P, T], bf16, tag="x2bf")
            with tc.tile_pool(name="pd", bufs=2) as pd, \
                 tc.tile_pool(name="pd_ps", bufs=2, space="PSUM") as pd_ps, \
                 tc.tile_pool(name="pd_st", bufs=2, space="PSUM") as pd_st:
                for t in range(NT):
                    sl = slice(t * TC, (t + 1) * TC)
                    ps = pd_ps.tile([P, TC], f32, tag="mm")
                    for k in range(8):
                        nc.tensor.matmul(ps[:], lhsT=wo_sb[:, k * P:(k + 1) * P],
                                         rhs=attnf[k][:, sl],
                                         start=(k == 0), stop=(k == 7))
                    gl = pd.tile([P, TC], bf16, tag="gl")
                    nc.vector.tensor_sub(gl[:], g_sb[:, 1 + t * TC: 1 + (t + 1) * TC],
                                         g_sb[:, t * TC:(t + 1) * TC])
                    gate = pd.tile([P, TC], f32, tag="gate")
                    nc.scalar.activation(gate[:], gl[:], AF.Sigmoid, bias=bg_sb[:])
                    ga = pd.tile([P, TC], f32, tag="ga")
                    nc.vector.tensor_mul(ga[:], gate[:], ps[:])
                    nc.vector.tensor_add(x2s32[:, sl], x1s32[:, sl], ga[:])
                    # LN stats for this chunk
                    nc.scalar.activation(x2bf[:, sl], x2s32[:, sl], AF.Copy)
                    x2sq = pd.tile([P, TC], bf16, tag="x2sq")
                    nc.scalar.activation(x2sq[:], x2bf[:, sl], AF.Square)
                    sp1 = pd_st.tile([1, TC], f32, tag="stat1")
                    nc.tensor.matmul(sp1[:], lhsT=ones128[:], rhs=x2bf[:, sl],
                                     start=True, stop=True)
                    sp2 = pd_st.tile([1, TC], f32, tag="stat2")
                    nc.tensor.matmul(sp2[:], lhsT=ones128[:], rhs=x2sq[:],
                                     start=True, stop=True)
                    st1 = pd.tile([1, TC], f32, tag="st1")
                    nc.scalar.activation(st1[:], sp1[:], AF.Copy)
                    st2 = pd.tile([1, TC], f32, tag="st2")
                    nc.scalar.activation(st2[:], sp2[:], AF.Copy)
                    nc.scalar.dma_start(out=ar_in[t][0:1, :], in_=st1[:])
                    nc.scalar.dma_start(out=ar_in[t][1:2, :], in_=st2[:])
                    nc.gpsimd.collective_compute(
                        "AllReduce", mybir.AluOpType.add, replica_groups=rg,
                        ins=[ar_in[t][:]], outs=[ar_out[t][:]])
            wx1_stack.close()

            # =================== Phase E: LayerNorm apply =================
            h_sh = late.tile([P, T], bf16, tag="h_sh")
            hf = attnf  # gathered h reuses x1f tiles (per-chunk columns)
            with tc.tile_pool(name="pe", bufs=2) as pe, \
                 tc.tile_pool(name="pe_ps", bufs=2, space="PSUM") as pe_ps:
                for t in range(NT):
                    sl = slice(t * TC, (t + 1) * TC)
                    s1row = pe.tile([1, TC], f32, tag="s1row")
                    nc.sync.dma_start(out=s1row[:], in_=ar_out[t][0:1, :])
                    s2row = pe.tile([1, TC], f32, tag="s2row")
                    nc.sync.dma_start(out=s2row[:], in_=ar_out[t][1:2, :])
                    s1b = pe_ps.tile([P, TC], f32, tag="s1b")
                    nc.tensor.matmul(s1b[:], lhsT=ones1f[:], rhs=s1row[:],
                                     start=True, stop=True)
                    s2b = pe_ps.tile([P, TC], f32, tag="s2b")
                    nc.tensor.matmul(s2b[:], lhsT=ones1f[:], rhs=s2row[:],
                                     start=True, stop=True)
                    # hp = x2 - mu ; mu2 = (s1b/D)^2 ; var = s2b/D - mu2
                    hp = pe.tile([P, TC], f32, tag="hp")
                    nc.vector.scalar_tensor_tensor(
                        out=hp[:], in0=s1b[:], scalar=-1.0 / D,
                        in1=x2s32[:, sl], op0=OP.mult, op1=OP.add)
                    mu2 = pe.tile([P, TC], f32, tag="mu2")
                    nc.scalar.activation(mu2[:], s1b[:], AF.Square, scale=1.0 / D)
                    var = pe.tile([P, TC], f32, tag="var")
                    nc.vector.scalar_tensor_tensor(
                        out=var[:], in0=s2b[:], scalar=1.0 / D,
                        in1=mu2[:], op0=OP.mult, op1=OP.subtract)
                    sd = pe.tile([P, TC], f32, tag="sd")
                    nc.scalar.activation(sd[:], var[:], AF.Sqrt, bias=eps128[:])
                    rstd = pe.tile([P, TC], f32, tag="rstd")
                    nc.vector.reciprocal(rstd[:], sd[:])
                    h2 = pe.tile([P, TC], f32, tag="h2")
                    nc.vector.tensor_mul(h2[:], hp[:], rstd[:])
                    nc.vector.tensor_scalar(
                        out=h_sh[:, sl], in0=h2[:], scalar1=lng_sb[:],
                        scalar2=lnb_sb[:], op0=OP.mult, op1=OP.add)
                    nc.scalar.dma_start(out=ag3_in[t][:], in_=h_sh[:, sl])
                    nc.gpsimd.collective_compute(
                        "AllGather", mybir.AluOpType.bypass, replica_groups=rg,
                        ins=[ag3_in[t][:]], outs=[ag3_out[t][:]])
                    for i in range(8):
                        nc.sync.dma_start(out=hf[i][:, sl],
                                          in_=ag3_out[t][i * P:(i + 1) * P, :])

            # =================== Phase F: MoE =============================
            with tc.tile_pool(name="pf", bufs=2) as pf, \
                 tc.tile_pool(name="pf_hid", bufs=2) as pf_hid, \
                 tc.tile_pool(name="pf_ps", bufs=2, space="PSUM") as pf_ps, \
                 tc.tile_pool(name="pf_ps2", bufs=2, space="PSUM") as pf_ps2, \
                 tc.tile_pool(name="pf_gw", bufs=1, space="PSUM") as pf_gw:
                for t in range(NT):
                    sl = slice(t * TC, (t + 1) * TC)
                    # gate weight row for this core's expert
                    lg = pf_gw.tile([4, TC], f32, tag="lg")
                    for k in range(8):
                        nc.tensor.matmul(lg[:], lhsT=wmg_sb[:, k * 4:(k + 1) * 4],
                                         rhs=hf[k][:, sl],
                                         start=(k == 0), stop=(k == 7))
                    gx = pf.tile([4, TC], bf16, tag="gx")
                    nc.scalar.activation(gx[:], lg[:], AF.Exp)
                    sm = pf_gw.tile([1, TC], f32, tag="sm")
                    nc.tensor.matmul(sm[:], lhsT=ones4[:], rhs=gx[:],
                                     start=True, stop=True)
                    sel = pf_gw.tile([1, TC], f32, tag="sel")
                    nc.tensor.matmul(sel[:], lhsT=esel_sb[:], rhs=gx[:],
                                     start=True, stop=True)
                    rc = pf.tile([1, TC], f32, tag="rc")
                    nc.vector.reciprocal(rc[:], sm[:])
                    gwrow = pf.tile([1, TC], f32, tag="gwrow")
                    nc.vector.tensor_mul(gwrow[:], sel[:], rc[:])
                    gwb_ps = pf_gw.tile([P, TC], f32, tag="gwb_ps")
                    nc.tensor.matmul(gwb_ps[:], lhsT=ones1f[:], rhs=gwrow[:],
                                     start=True, stop=True)
                    gwb = pf.tile([P, TC], f32, tag="gwb")
                    nc.scalar.activation(gwb[:], gwb_ps[:], AF.Copy)

                    # hid = silu(w1 @ h + b1)
                    hid = [pf_hid.tile([P, TC], bf16, tag=f"hid{ft}",
                                       name=f"hid{ft}") for ft in range(16)]
                    for ft in range(16):
                        hp = pf_ps.tile([P, TC], f32, tag="hid")
                        for k in range(8):
                            nc.tensor.matmul(
                                hp[:],
                                lhsT=w1_sb[k][:, ft * P:(ft + 1) * P],
                                rhs=hf[k][:, sl], start=(k == 0), stop=(k == 7))
                        sg = pf.tile([P, TC], bf16, tag="sg")
                        nc.scalar.activation(sg[:], hp[:], AF.Sigmoid,
                                             bias=b1s_sb[:, ft:ft + 1])
                        nc.vector.scalar_tensor_tensor(
                            out=hid[ft][:], in0=hp[:], scalar=b1s_sb[:, ft:ft + 1],
                            in1=sg[:], op0=OP.add, op1=OP.mult)
                    for dt in range(8):
                        op = pf_ps2.tile([P, TC], f32, tag="out")
                        for ft in range(16):
                            nc.tensor.matmul(
                                op[:],
                                lhsT=w2_sb[ft][:, dt * P:(dt + 1) * P],
                                rhs=hid[ft][:], start=(ft == 0), stop=False,
                                skip_group_check=True)
                        nc.tensor.matmul(op[:], lhsT=b2s_sb[:, dt * P:(dt + 1) * P],
                                         rhs=ones512[:], start=False, stop=True,
                                         skip_group_check=True)
                        par = pf.tile([P, TC], bf16, tag="par")
                        nc.vector.tensor_mul(par[:], op[:], gwb[:])
                        nc.scalar.dma_start(out=rs_in[t][dt * P:(dt + 1) * P, :],
                                            in_=par[:])
                    nc.gpsimd.collective_compute(
                        "ReduceScatter", mybir.AluOpType.add, replica_groups=rg,
                        ins=[rs_in[t][:]], outs=[rs_out[t][:]])
                    fo = pf.tile([P, TC], bf16, tag="fo")
                    nc.sync.dma_start(out=fo[:], in_=rs_out[t][:])
                    yo = pf.tile([P, TC], f32, tag="yo")
                    nc.vector.tensor_add(yo[:], x2s32[:, sl], fo[:])
                    nc.scalar.dma_start(out=y_d[:, sl], in_=yo[:])
            late_stack.close()
    nc.finalize()
    return nc


def _prep_inputs(inputs):
    x = np.asarray(inputs["x"])[0]          # [T, D] f32
    w_dw = np.asarray(inputs["w_dw"])
    w_pw = np.asarray(inputs["w_pw"])
    w_qkv = np.asarray(inputs["w_qkv"])
    w_o = np.asarray(inputs["w_o"])
    w_gate = np.asarray(inputs["w_gate"])
    b_gate = np.asarray(inputs["b_gate"])
    ln_g = np.asarray(inputs["ln_g"])
    ln_b = np.asarray(inputs["ln_b"])
    w_mg = np.asarray(inputs["w_moe_gate"])
    w1 = np.asarray(inputs["w1"])
    b1 = np.asarray(inputs["b1"])
    w2 = np.asarray(inputs["w2"])
    b2 = np.asarray(inputs["b2"])

    xT = np.ascontiguousarray(x.T)                       # [D, T]
    xtp = np.zeros((D, T + 2), dtype=BF)
    xtp[:, 2:] = xT.astype(BF)

    # depthwise taps per partition: wdw[p, 3k+j] = w_dw[k*128+p, j]
    wdw = np.ascontiguousarray(
        w_dw.reshape(8, P, 3).transpose(1, 0, 2).reshape(P, 24)
    ).astype(np.float32)

    # rope tables
    inv_freq = 1.0 / (10000.0 ** (np.arange(0, DH, 2, dtype=np.float32) / DH))
    pos = np.arange(T, dtype=np.float32)
    theta = pos[None, :] * inv_freq[:, None]             # [32, T]
    cos64 = np.concatenate([np.cos(theta), np.cos(theta)], axis=0)
    sin64 = np.concatenate([-np.sin(theta), np.sin(theta)], axis=0)
    ctab = np.tile(cos64, (2, 1)).astype(BF)             # [128, T]
    stab = np.tile(sin64, (2, 1)).astype(BF)

    p64 = np.zeros((P, P), dtype=BF)
    for r in range(P):
        p64[r, (r % 64 + 32) % 64 + 64 * (r // 64)] = 1.0
    # p64 is used as lhsT: out[i,t] = sum_k p64[k,i] q[k,t] = q[swap(i),t]

    mask = np.triu(np.ones((P, P), np.float32))          # [s, t] keep s<=t

    perm = np.concatenate([np.arange(0, DH, 2), np.arange(1, DH, 2)])
    wq, wk, wv = w_qkv[0:D], w_qkv[D:2 * D], w_qkv[2 * D:3 * D]

    in_maps = []
    for c in range(NC_N):
        heads = [2 * c, 2 * c + 1]
        qrows = np.concatenate([h * DH + perm for h in heads])
        krows = qrows
        vrows = np.concatenate([np.arange(h * DH, (h + 1) * DH) for h in heads])
        wqkvT = np.concatenate(
            [wq[qrows].T, wk[krows].T, wv[vrows].T], axis=1)   # [1024, 384]
        e, hh = c // 2, c % 2
        w1s = w1[e, hh * 2048:(hh + 1) * 2048, :].T          # [1024, 2048]
        b1s = b1[e, hh * 2048:(hh + 1) * 2048]               # [2048]
        w2s = w2[e, :, hh * 2048:(hh + 1) * 2048].T          # [2048, 1024]
        b2s = (b2[e] if hh == 0 else np.zeros(D, np.float32))
        esel = np.zeros((4, 1), dtype=BF)
        esel[e, 0] = 1.0

        w2s_packed = np.empty((P, 16 * 1024), dtype=BF)
        for k in range(16):
            w2s_packed[:, k * 1024:(k + 1) * 1024] = \
                w2s[k * P:(k + 1) * P].astype(BF)
        b1sp = b1s.reshape(16, P).T.astype(np.float32)
        in_maps.append({
            "xtp": xtp,
            "xs32": np.ascontiguousarray(xT[c * P:(c + 1) * P]).astype(np.float32),
            "wdw": wdw,
            "wpw": _pack_k(w_pw.T[:, c * P:(c + 1) * P].astype(BF), P),
            "wqkv": _pack_k(wqkvT.astype(BF), 384),
            "ctab": ctab, "stab": stab, "p64": p64, "mask": mask,
            "wo": _pack_k(w_o.T[:, c * P:(c + 1) * P].astype(BF), P),
            "wg": _pack_k(w_gate.T[:, c * P:(c + 1) * P].astype(BF), P),
            "bg": b_gate[c * P:(c + 1) * P].reshape(P, 1).astype(np.float32),
            "lng": ln_g[c * P:(c + 1) * P].reshape(P, 1).astype(np.float32),
            "lnb": ln_b[c * P:(c + 1) * P].reshape(P, 1).astype(np.float32),
            "wmg": _pack_k(w_mg.T.astype(BF), 4),
            "esel": esel,
            "w1s": _pack_k(w1s.astype(BF), 2048),
            "b1s": b1sp,
            "w2s": w2s_packed,
            "b2s": b2s.reshape(1, D).astype(BF),
        })
    return in_maps


def _pack_k(mat_km, M, rot=0):
    """[1024, M] -> [128, 8*M]: k-block i holds rows of block (rot+i)%8.

    rot matches the conv out-block rotation: x1f[i] holds x1 rows
    ((rot+i)%8)*128.., so contraction block i must use those weight rows.
    """
    out = np.empty((P, 8 * M), dtype=mat_km.dtype)
    for k in range(8):
        o = (rot + k) % 8
        out[:, k * M:(k + 1) * M] = mat_km[o * P:(o + 1) * P]
    return out


def kernel(**inputs) -> np.ndarray:
    global _PROGRAM
    from concourse.bass_utils import run_bass_kernel_spmd

    if _PROGRAM is None:
        _PROGRAM = _build_program()
    nc = _PROGRAM
    in_maps = _prep_inputs(inputs)
    last_err = None
    for _attempt in range(2):
        try:
            res = run_bass_kernel_spmd(nc, in_maps, list(range(NC_N)))
            break
        except Exception as exc:  # transient device hiccups: retry once
            last_err = exc
    else:
        raise last_err
    outT = np.empty((D, T), dtype=np.float32)
    for c in range(NC_N):
        outT[c * P:(c + 1) * P] = res.results[c]["y"]
    return np.ascontiguousarray(outT.T)[None, :, :].astype(np.float32)


# revision 50
# speedup vs baseline: 1.1170x; 1.0098x over previous
"""Trainium2 Bass kernel for nn_DeltaNet_19430432047178.

Strategy (8 cores, SPMD):
  - activations live transposed on device: [d, T] with d on partitions
  - depthwise conv k=3 computed on the vector engine (3 shifted
    per-partition FMAs); pointwise conv is a single row-sharded matmul
  - attention tensor-parallel over heads (2 heads/core), chunked linear
    attention (C=128) with KV state accumulated in PSUM
  - w_o / delta-gate / LN row-sharded over d
  - MoE sharded expert x hidden-half (core c -> expert c//2, half c%2)
  - all collectives are chunked over T (4 chunks of 512) and pipelined
    under compute: 4x AG(x1), 4x AG(attn), 4x AR(LN stats), 4x AG(h),
    4x RS(ffn partial, bf16)
  - gpsimd issues only collectives; data DMAs ride sync/scalar (HWDGE)
  - all matmuls bf16 with f32 PSUM accumulation
"""
import numpy as np
import ml_dtypes


NC_N = 8
T = 2048
D = 1024
H = 16
DH = 64
E = 4
HD = 4096
P = 128
CH = 128            # attention chunk
NCH = T // CH       # 16
NT = 4              # T chunks of 512 for GEMMs and collectives
TC = 512

BF = ml_dtypes.bfloat16

_PROGRAM = None  # cached nc


def _build_program():
    import concourse.mybir as mybir
    import concourse.tile as tile
    from concourse import bacc
    from concourse.masks import make_identity

    f32 = mybir.dt.float32
    bf16 = mybir.dt.bfloat16
    AF = mybir.ActivationFunctionType
    OP = mybir.AluOpType

    nc = bacc.Bacc()

    # ---------------- external params (per-core) ----------------
    xtp_d = nc.declare_dram_parameter("xtp", [D, T + 2], bf16, isOutput=False)
    xs32_d = nc.declare_dram_parameter("xs32", [P, T], f32, isOutput=False)
    wdw_d = nc.declare_dram_parameter("wdw", [P, 24], f32, isOutput=False)
    wpw_d = nc.declare_dram_parameter("wpw", [P, 8 * P], bf16, isOutput=False)
    wqkv_d = nc.declare_dram_parameter("wqkv", [P, 8 * 384], bf16, isOutput=False)
    ctab_d = nc.declare_dram_parameter("ctab", [P, T], bf16, isOutput=False)
    stab_d = nc.declare_dram_parameter("stab", [P, T], bf16, isOutput=False)
    p64_d = nc.declare_dram_parameter("p64", [P, P], bf16, isOutput=False)
    mask_d = nc.declare_dram_parameter("mask", [P, P], f32, isOutput=False)
    wo_d = nc.declare_dram_parameter("wo", [P, 8 * P], bf16, isOutput=False)
    wg_d = nc.declare_dram_parameter("wg", [P, 8 * P], bf16, isOutput=False)
    bg_d = nc.declare_dram_parameter("bg", [P, 1], f32, isOutput=False)
    lng_d = nc.declare_dram_parameter("lng", [P, 1], f32, isOutput=False)
    lnb_d = nc.declare_dram_parameter("lnb", [P, 1], f32, isOutput=False)
    wmg_d = nc.declare_dram_parameter("wmg", [P, 8 * 4], bf16, isOutput=False)
    esel_d = nc.declare_dram_parameter("esel", [4, 1], bf16, isOutput=False)
    w1s_d = nc.declare_dram_parameter("w1s", [P, 8 * 2048], bf16, isOutput=False)
    b1s_d = nc.declare_dram_parameter("b1s", [P, 16], f32, isOutput=False)
    w2s_d = nc.declare_dram_parameter("w2s", [P, 16 * 1024], bf16, isOutput=False)
    b2s_d = nc.declare_dram_parameter("b2s", [1, 1024], bf16, isOutput=False)
    y_d = nc.declare_dram_parameter("y", [P, T], f32, isOutput=True)

    # ---------------- internal DRAM (chunked collectives) ----------------
    rg = [list(range(NC_N))]
    dum_in = nc.dram_tensor("dum_in", [P, 16], bf16)
    dum_out = nc.dram_tensor("dum_out", [D, 16], bf16, addr_space="Shared")
    ag1_in = [nc.dram_tensor(f"ag1_in{t}", [P, TC], bf16) for t in range(NT)]
    ag1_out = [nc.dram_tensor(f"ag1_out{t}", [D, TC], bf16, addr_space="Shared")
               for t in range(NT)]
    ag2_in = [nc.dram_tensor(f"ag2_in{t}", [P, TC], bf16) for t in range(NT)]
    ag2_out = [nc.dram_tensor(f"ag2_out{t}", [D, TC], bf16, addr_space="Shared")
               for t in range(NT)]
    ar_in = [nc.dram_tensor(f"ar_in{t}", [2, TC], f32) for t in range(NT)]
    ar_out = [nc.dram_tensor(f"ar_out{t}", [2, TC], f32, addr_space="Shared")
              for t in range(NT)]
    ag3_in = [nc.dram_tensor(f"ag3_in{t}", [P, TC], bf16) for t in range(NT)]
    ag3_out = [nc.dram_tensor(f"ag3_out{t}", [D, TC], bf16, addr_space="Shared")
               for t in range(NT)]
    rs_in = [nc.dram_tensor(f"rs_in{t}", [D, TC], bf16) for t in range(NT)]
    rs_out = [nc.dram_tensor(f"rs_out{t}", [P, TC], bf16) for t in range(NT)]

    import concourse.bass as bass
    import contextlib

    with tile.TileContext(nc) as tc:
        with contextlib.ExitStack() as stack:
            consts = stack.enter_context(tc.tile_pool(name="consts", bufs=1))
            late_stack = contextlib.ExitStack()
            late = late_stack.enter_context(tc.tile_pool(name="late", bufs=1))
            x2s32 = late.tile([P, T], f32, tag="x2s32")
            x1f = [late.tile([P, T], bf16, tag=f"x1f{i}", name=f"x1f{i}")
                   for i in range(8)]
            wx1_stack = contextlib.ExitStack()
            wx1 = wx1_stack.enter_context(tc.tile_pool(name="wx1", bufs=1))
            x1s32 = wx1.tile([P, T], f32, tag="x1s32")
            g_sb = wx1.tile([P, T + 1], bf16, tag="g_sb")
            xtp_stack = contextlib.ExitStack()
            xtp_pool = xtp_stack.enter_context(tc.tile_pool(name="xtp", bufs=1))
            xtp = [xtp_pool.tile([P, T + 2], bf16, tag=f"xtp{i}", name=f"xtp{i}")
                   for i in range(8)]
            # x DMAs first: conv (the only collective-free compute) starts ASAP
            for k in range(8):
                eng = nc.scalar if k % 2 == 0 else nc.sync
                eng.dma_start(out=xtp[k][:], in_=xtp_d[k * P:(k + 1) * P, :])

            # ---- constants / weights ----
            wdw_sb = consts.tile([P, 24], f32, tag="wdw")
            nc.scalar.dma_start(out=wdw_sb[:], in_=wdw_d[:])
            wpw_sb = consts.tile([P, 8 * P], bf16, tag="wpw")
            nc.scalar.dma_start(out=wpw_sb[:], in_=wpw_d[:])
            wqkv_sb = consts.tile([P, 8 * 384], bf16, tag="wqkv")
            nc.scalar.dma_start(out=wqkv_sb[:], in_=wqkv_d[:])
            wo_sb = consts.tile([P, 8 * P], bf16, tag="wo")
            nc.scalar.dma_start(out=wo_sb[:], in_=wo_d[:])
            wg_sb = consts.tile([P, 8 * P], bf16, tag="wg")
            nc.scalar.dma_start(out=wg_sb[:], in_=wg_d[:])
            bg_sb = consts.tile([P, 1], f32, tag="bg")
            nc.scalar.dma_start(out=bg_sb[:], in_=bg_d[:])
            lng_sb = consts.tile([P, 1], f32, tag="lng")
            nc.scalar.dma_start(out=lng_sb[:], in_=lng_d[:])
            lnb_sb = consts.tile([P, 1], f32, tag="lnb")
            nc.scalar.dma_start(out=lnb_sb[:], in_=lnb_d[:])
            wmg_sb = consts.tile([P, 8 * 4], bf16, tag="wmg")
            nc.scalar.dma_start(out=wmg_sb[:], in_=wmg_d[:])
            esel_sb = consts.tile([4, 1], bf16, tag="esel")
            nc.scalar.dma_start(out=esel_sb[:], in_=esel_d[:])
            b1s_sb = consts.tile([P, 16], f32, tag="b1s")
            nc.scalar.dma_start(out=b1s_sb[:], in_=b1s_d[:])
            b2s_sb = consts.tile([1, 1024], bf16, tag="b2s")
            nc.scalar.dma_start(out=b2s_sb[:], in_=b2s_d[:])
            ctab_sb = consts.tile([P, T], bf16, tag="ctab")
            nc.scalar.dma_start(out=ctab_sb[:], in_=ctab_d[:])
            stab_sb = consts.tile([P, T], bf16, tag="stab")
            nc.scalar.dma_start(out=stab_sb[:], in_=stab_d[:])
            p64_sb = consts.tile([P, P], bf16, tag="p64")
            nc.scalar.dma_start(out=p64_sb[:], in_=p64_d[:])
            mask_sb = consts.tile([P, P], f32, tag="mask")
            nc.scalar.dma_start(out=mask_sb[:], in_=mask_d[:])
            ident_sb = consts.tile([P, P], bf16, tag="ident")
            make_identity(nc, ident_sb[:])
            ones128 = consts.tile([P, 1], bf16, tag="ones128")
            nc.vector.memset(ones128[:], 1.0)
            ones4 = consts.tile([4, 1], bf16, tag="ones4")
            nc.vector.memset(ones4[:], 1.0)
            ones512 = consts.tile([1, TC], bf16, tag="ones512")
            nc.vector.memset(ones512[:], 1.0)
            ones1f = consts.tile([1, P], f32, tag="ones1f")
            nc.vector.memset(ones1f[:], 1.0)
            eps128 = consts.tile([P, 1], f32, tag="eps128")
            nc.vector.memset(eps128[:], 1e-5)

            # warm up collectives (barrier + first-RDH cost) while DMAs run
            dum_sb = consts.tile([P, 16], bf16, tag="dum")
            nc.vector.memset(dum_sb[:], 0.0)
            nc.scalar.dma_start(out=dum_in[:], in_=dum_sb[:])
            nc.gpsimd.collective_compute(
                "AllGather", mybir.AluOpType.bypass, replica_groups=rg,
                ins=[dum_in[:]], outs=[dum_out[:]])

            # MoE weights prefetch (sync queue; needed from ~mid-kernel)
            w1_sb = [consts.tile([P, 2048], bf16, tag=f"w1_{k}", name=f"w1_{k}")
                     for k in range(8)]
            for k in range(8):
                nc.sync.dma_start(out=w1_sb[k][:],
                                  in_=w1s_d[:, k * 2048:(k + 1) * 2048])
            w2_sb = [consts.tile([P, 1024], bf16, tag=f"w2_{k}", name=f"w2_{k}")
                     for k in range(16)]
            for k in range(16):
                nc.sync.dma_start(out=w2_sb[k][:],
                                  in_=w2s_d[:, k * 1024:(k + 1) * 1024])

            # persistent activations
            # =================== Phase A: depthwise (DVE) + pointwise ====
            with tc.tile_pool(name="pa", bufs=2) as pa, \
                 tc.tile_pool(name="dwy", bufs=2) as dwy_pool, \
                 tc.tile_pool(name="pa_ps", bufs=3, space="PSUM") as pa_ps:
                for t in range(NT):
                    sl = slice(t * TC, (t + 1) * TC)
                    xs32_sb = pa.tile([P, TC], f32, tag="xs32")
                    nc.scalar.dma_start(out=xs32_sb[:], in_=xs32_d[:, sl])
                    ps = pa_ps.tile([P, TC], f32, tag="mm")
                    for k in range(8):
                        # dwy = sum_j xtp[k][:, t*TC+j : +TC] * wdw[:, 3k+j]
                        d = dwy_pool.tile([P, TC], bf16, tag=f"dwy{k}")
                        nc.vector.tensor_scalar_mul(
                            d[:], xtp[k][:, t * TC:t * TC + TC],
                            wdw_sb[:, 3 * k:3 * k + 1])
                        nc.vector.scalar_tensor_tensor(
                            out=d[:], in0=xtp[k][:, t * TC + 1:t * TC + 1 + TC],
                            scalar=wdw_sb[:, 3 * k + 1:3 * k + 2], in1=d[:],
                            op0=OP.mult, op1=OP.add)
                        nc.vector.scalar_tensor_tensor(
                            out=d[:], in0=xtp[k][:, t * TC + 2:t * TC + 2 + TC],
                            scalar=wdw_sb[:, 3 * k + 2:3 * k + 3], in1=d[:],
                            op0=OP.mult, op1=OP.add)
                        nc.tensor.matmul(
                            ps[:], lhsT=wpw_sb[:, k * P:(k + 1) * P],
                            rhs=d[:],
                            start=(k == 0), stop=(k == 7))
                    nc.vector.tensor_add(x1s32[:, sl], xs32_sb[:], ps[:])
                    x1bf = pa.tile([P, TC], bf16, tag="x1bf")
                    nc.scalar.activation(x1bf[:], x1s32[:, sl], AF.Copy)
                    nc.scalar.dma_start(out=ag1_in[t][:], in_=x1bf[:])
                    nc.gpsimd.collective_compute(
                        "AllGather", mybir.AluOpType.bypass, replica_groups=rg,
                        ins=[ag1_in[t][:]], outs=[ag1_out[t][:]])
                    for i in range(8):
                        nc.sync.dma_start(out=x1f[i][:, sl],
                                          in_=ag1_out[t][i * P:(i + 1) * P, :])
            xtp_stack.close()

            # =================== Phase B: qkv + rope + phi + gate logits =
            bc_stack = contextlib.ExitStack()
            bc = bc_stack.enter_context(tc.tile_pool(name="bc", bufs=1))
            qphi = bc.tile([P, T], bf16, tag="qphi")
            kphi = bc.tile([P, T], bf16, tag="kphi")
            vbf = bc.tile([P, T], bf16, tag="vbf")
            with tc.tile_pool(name="pb", bufs=2) as pb, \
                 tc.tile_pool(name="pb_ps", bufs=3, space="PSUM") as pb_ps:
                for t in range(NT):
                    sl = slice(t * TC, (t + 1) * TC)
                    for which, dst in ((0, qphi), (1, kphi)):
                        ps = pb_ps.tile([P, TC], f32, tag="mm")
                        for k in range(8):
                            nc.tensor.matmul(
                                ps[:],
                                lhsT=wqkv_sb[:, k * 384 + which * P:
                                             k * 384 + which * P + P],
                                rhs=x1f[k][:, sl],
                                start=(k == 0), stop=(k == 7))
                        qc = pb.tile([P, TC], bf16, tag="qc")
                        nc.scalar.activation(qc[:], ps[:], AF.Copy)
                        ps2 = pb_ps.tile([P, TC], f32, tag="mm")
                        nc.tensor.matmul(ps2[:], lhsT=p64_sb[:], rhs=qc[:],
                                         start=True, stop=True)
                        qsw = pb.tile([P, TC], bf16, tag="qsw")
                        nc.scalar.activation(qsw[:], ps2[:], AF.Copy)
                        t1 = pb.tile([P, TC], bf16, tag="t1")
                        nc.vector.tensor_mul(t1[:], qc[:], ctab_sb[:, sl])
                        t2 = pb.tile([P, TC], bf16, tag="t2")
                        nc.vector.tensor_mul(t2[:], qsw[:], stab_sb[:, sl])
                        qr = pb.tile([P, TC], bf16, tag="qr")
                        nc.vector.tensor_add(qr[:], t1[:], t2[:])
                        # phi = min(exp(qr),1) + relu(qr)
                        ex = pb.tile([P, TC], bf16, tag="ex")
                        nc.scalar.activation(ex[:], qr[:], AF.Exp)
                        rl = pb.tile([P, TC], bf16, tag="rl")
                        nc.vector.tensor_scalar_max(rl[:], qr[:], 0.0)
                        nc.vector.scalar_tensor_tensor(
                            out=dst[:, sl], in0=ex[:], scalar=1.0, in1=rl[:],
                            op0=OP.min, op1=OP.add)
                    # v projection
                    ps = pb_ps.tile([P, TC], f32, tag="mm")
                    for k in range(8):
                        nc.tensor.matmul(
                            ps[:],
                            lhsT=wqkv_sb[:, k * 384 + 2 * P: k * 384 + 3 * P],
                            rhs=x1f[k][:, sl],
                            start=(k == 0), stop=(k == 7))
                    nc.scalar.activation(vbf[:, sl], ps[:], AF.Copy)
                    # gate logits G = W_g @ x1 (x1f chunk freed for attnf after)
                    ps = pb_ps.tile([P, TC], f32, tag="mm")
                    for k in range(8):
                        nc.tensor.matmul(ps[:], lhsT=wg_sb[:, k * P:(k + 1) * P],
                                         rhs=x1f[k][:, sl],
                                         start=(k == 0), stop=(k == 7))
                    nc.scalar.activation(g_sb[:, 1 + t * TC: 1 + (t + 1) * TC],
                                         ps[:], AF.Copy)
                    if t == 0:
                        nc.scalar.activation(g_sb[:, 0:1], ps[:, 0:1], AF.Copy)

            # =================== Phase C: chunked linear attention ========
            attn_sh = bc.tile([P, T], bf16, tag="attn_sh")
            attnf = x1f  # gathered attention reuses x1f (per-chunk columns)
            with tc.tile_pool(name="pc", bufs=2) as pc, \
                 tc.tile_pool(name="pc_ps", bufs=2, space="PSUM") as pc_ps, \
                 tc.tile_pool(name="pc_o", bufs=2, space="PSUM") as pc_o, \
                 tc.tile_pool(name="pc_num", bufs=2, space="PSUM") as pc_num:
                # running KV state lives in SBUF; updated by DVE so the
                # serial inter-chunk chain never waits on the PE block
                s_run = bc.tile([P, 65], f32, tag="s_run")
                nc.vector.memset(s_run[:], 0.0)
                for ci in range(NCH):
                    sl = slice(ci * CH, (ci + 1) * CH)
                    # token-major copies of k, v
                    tp1 = pc_ps.tile([P, P], bf16, tag="tp")
                    nc.tensor.transpose(tp1[:], kphi[:, sl], ident_sb[:])
                    ktok = pc.tile([P, P], bf16, tag="ktok")
                    nc.scalar.activation(ktok[:], tp1[:], AF.Copy)
                    tp2 = pc_ps.tile([P, P], bf16, tag="tp")
                    nc.tensor.transpose(tp2[:], vbf[:, sl], ident_sb[:])
                    vtok = pc.tile([P, P], bf16, tag="vtok")
                    nc.scalar.activation(vtok[:], tp2[:], AF.Copy)
                    if ci > 0:
                        kv_sb = pc.tile([P, 65], bf16, tag="kvsb")
                        nc.vector.tensor_scalar_add(kv_sb[:], s_run[:], 0.0)
                    # this chunk's outer product (issued early in the PE block)
                    o_ps = pc_o.tile([P, 65], f32, tag="o")
                    nc.tensor.matmul(o_ps[0:64, 0:64], lhsT=ktok[:, 0:64],
                                     rhs=vtok[:, 0:64], start=True, stop=True,
                                     skip_group_check=True)
                    nc.tensor.matmul(o_ps[64:128, 0:64], lhsT=ktok[:, 64:128],
                                     rhs=vtok[:, 64:128], start=True, stop=True,
                                     tile_position=(0, 64), skip_group_check=True)
                    nc.tensor.matmul(o_ps[:, 64:65], lhsT=ktok[:],
                                     rhs=ones128[:], start=True, stop=True,
                                     skip_group_check=True)
                    nc.vector.tensor_add(s_run[:], s_run[:], o_ps[:])
                    atn = pc.tile([P, P], bf16, tag="atn")
                    nm = pc_num.tile([P, 130], f32, tag="num")
                    for h in (0, 1):
                        hs = slice(64 * h, 64 * h + 64)
                        ns = slice(65 * h, 65 * h + 64)
                        st_ps = pc_ps.tile([P, P], f32, tag="mm")
                        nc.tensor.matmul(st_ps[:], lhsT=kphi[hs, sl],
                                         rhs=qphi[hs, sl], start=True, stop=True)
                        stm = pc.tile([P, P], bf16, tag="stm")
                        nc.vector.tensor_mul(stm[:], st_ps[:], mask_sb[:])
                        nc.tensor.matmul(nm[:, ns], lhsT=stm[:],
                                         rhs=vtok[:, hs], start=(h == 0),
                                         stop=(ci == 0), skip_group_check=True)
                        nc.tensor.matmul(nm[:, 65 * h + 64:65 * h + 65], lhsT=stm[:],
                                         rhs=ones128[:], start=False,
                                         stop=(ci == 0), skip_group_check=True)
                        if ci > 0:
                            nc.tensor.matmul(nm[:, ns], lhsT=qphi[hs, sl],
                                             rhs=kv_sb[hs, 0:64], start=False,
                                             stop=True, skip_group_check=True)
                            nc.tensor.matmul(nm[:, 65 * h + 64:65 * h + 65],
                                             lhsT=qphi[hs, sl],
                                             rhs=kv_sb[hs, 64:65], start=False,
                                             stop=True, skip_group_check=True)
                    den2 = pc.tile([P, 2], f32, tag="den2")
                    den_ap = bass.AP(tensor=nm.tensor, offset=nm.offset + 64,
                                     ap=[list(nm.ap[0]), [65, 2]])
                    nc.vector.tensor_scalar_add(den2[:], den_ap, 1e-6)
                    nc.vector.reciprocal(den2[:], den2[:])
                    for h in (0, 1):
                        nc.vector.tensor_scalar_mul(
                            atn[:, 64 * h:64 * h + 64], nm[:, 65 * h:65 * h + 64],
                            den2[:, h:h + 1])
                    tp3 = pc_ps.tile([P, P], bf16, tag="tp")
                    nc.tensor.transpose(tp3[:], atn[:], ident_sb[:])
                    nc.scalar.activation(attn_sh[:, sl], tp3[:], AF.Copy)
                    if ci % 4 == 3:
                        t = ci // 4
                        tsl = slice(t * TC, (t + 1) * TC)
                        nc.scalar.dma_start(out=ag2_in[t][:],
                                            in_=attn_sh[:, tsl])
                        nc.gpsimd.collective_compute(
                            "AllGather", mybir.AluOpType.bypass,
                            replica_groups=rg,
                            ins=[ag2_in[t][:]], outs=[ag2_out[t][:]])
                        for i in range(8):
                            nc.sync.dma_start(
                                out=attnf[i][:, tsl],
                                in_=ag2_out[t][i * P:(i + 1) * P, :])
            bc_stack.close()

            # =================== Phase D: w_o + gate + x2 + LN stats ======
            x2bf = late.tile([P, T], bf16, tag="x2bf")
            with tc.tile_pool(name="pd", bufs=2) as pd, \
                 tc.tile_pool(name="pd_ps", bufs=2, space="PSUM") as pd_ps, \
                 tc.tile_pool(name="pd_st", bufs=2, space="PSUM") as pd_st:
                for t in range(NT):
                    sl = slice(t * TC, (t + 1) * TC)
                    ps = pd_ps.tile([P, TC], f32, tag="mm")
                    for k in range(8):
                        nc.tensor.matmul(ps[:], lhsT=wo_sb[:, k * P:(k + 1) * P],
                                         rhs=attnf[k][:, sl],
                                         start=(k == 0), stop=(k == 7))
                    gl = pd.tile([P, TC], bf16, tag="gl")
                    nc.vector.tensor_sub(gl[:], g_sb[:, 1 + t * TC: 1 + (t + 1) * TC],
                                         g_sb[:, t * TC:(t + 1) * TC])
                    gate = pd.tile([P, TC], f32, tag="gate")
                    nc.scalar.activation(gate[:], gl[:], AF.Sigmoid, bias=bg_sb[:])
                    ga = pd.tile([P, TC], f32, tag="ga")
                    nc.vector.tensor_mul(ga[:], gate[:], ps[:])
                    nc.vector.tensor_add(x2s32[:, sl], x1s32[:, sl], ga[:])
                    # LN stats for this chunk
                    nc.scalar.activation(x2bf[:, sl], x2s32[:, sl], AF.Copy)
                    x2sq = pd.tile([P, TC], bf16, tag="x2sq")
                    nc.scalar.activation(x2sq[:], x2bf[:, sl], AF.Square)
                    sp1 = pd_st.tile([1, TC], f32, tag="stat1")
                    nc.tensor.matmul(sp1[:], lhsT=ones128[:], rhs=x2bf[:, sl],
                                     start=True, stop=True)
                    sp2 = pd_st.tile([1, TC], f32, tag="stat2")
                    nc.tensor.matmul(sp2[:], lhsT=ones128[:], rhs=x2sq[:],
                                     start=True, stop=True)
                    st1 = pd.tile([1, TC], f32, tag="st1")
                    nc.scalar.activation(st1[:], sp1[:], AF.Copy)
                    st2 = pd.tile([1, TC], f32, tag="st2")
                    nc.scalar.activation(st2[:], sp2[:], AF.Copy)
                    nc.scalar.dma_start(out=ar_in[t][0:1, :], in_=st1[:])
                    nc.scalar.dma_start(out=ar_in[t][1:2, :], in_=st2[:])
                    nc.gpsimd.collective_compute(
                        "AllReduce", mybir.AluOpType.add, replica_groups=rg,
                        ins=[ar_in[t][:]], outs=[ar_out[t][:]])
            wx1_stack.close()

            # =================== Phase E: LayerNorm apply =================
            h_sh = late.tile([P, T], bf16, tag="h_sh")
            hf = attnf  # gathered h reuses x1f tiles (per-chunk columns)
            with tc.tile_pool(name="pe", bufs=2) as pe, \
                 tc.tile_pool(name="pe_ps", bufs=2, space="PSUM") as pe_ps:
                for t in range(NT):
                    sl = slice(t * TC, (t + 1) * TC)
                    s1row = pe.tile([1, TC], f32, tag="s1row")
                    nc.sync.dma_start(out=s1row[:], in_=ar_out[t][0:1, :])
                    s2row = pe.tile([1, TC], f32, tag="s2row")
                    nc.sync.dma_start(out=s2row[:], in_=ar_out[t][1:2, :])
                    s1b = pe_ps.tile([P, TC], f32, tag="s1b")
                    nc.tensor.matmul(s1b[:], lhsT=ones1f[:], rhs=s1row[:],
                                     start=True, stop=True)
                    s2b = pe_ps.tile([P, TC], f32, tag="s2b")
                    nc.tensor.matmul(s2b[:], lhsT=ones1f[:], rhs=s2row[:],
                                     start=True, stop=True)
                    # hp = x2 - mu ; mu2 = (s1b/D)^2 ; var = s2b/D - mu2
                    hp = pe.tile([P, TC], f32, tag="hp")
                    nc.vector.scalar_tensor_tensor(
                        out=hp[:], in0=s1b[:], scalar=-1.0 / D,
                        in1=x2s32[:, sl], op0=OP.mult, op1=OP.add)
                    mu2 = pe.tile([P, TC], f32, tag="mu2")
                    nc.scalar.activation(mu2[:], s1b[:], AF.Square, scale=1.0 / D)
                    var = pe.tile([P, TC], f32, tag="var")
                    nc.vector.scalar_tensor_tensor(
                        out=var[:], in0=s2b[:], scalar=1.0 / D,
                        in1=mu2[:], op0=OP.mult, op1=OP.subtract)
                    sd = pe.tile([P, TC], f32, tag="sd")
                    nc.scalar.activation(sd[:], var[:], AF.Sqrt, bias=eps128[:])
                    rstd = pe.tile([P, TC], f32, tag="rstd")
                    nc.vector.reciprocal(rstd[:], sd[:])
                    h2 = pe.tile([P, TC], f32, tag="h2")
                    nc.vector.tensor_mul(h2[:], hp[:], rstd[:])
                    nc.vector.tensor_scalar(
                        out=h_sh[:, sl], in0=h2[:], scalar1=lng_sb[:],
                        scalar2=lnb_sb[:], op0=OP.mult, op1=OP.add)
                    nc.scalar.dma_start(out=ag3_in[t][:], in_=h_sh[:, sl])
                    nc.gpsimd.collective_compute(
                        "AllGather", mybir.AluOpType.bypass, replica_groups=rg,
                        ins=[ag3_in[t][:]], outs=[ag3_out[t][:]])
                    for i in range(8):
                        nc.sync.dma_start(out=hf[i][:, sl],
                                          in_=ag3_out[t][i * P:(i + 1) * P, :])

            # =================== Phase F: MoE =============================
            with tc.tile_pool(name="pf", bufs=2) as pf, \
                 tc.tile_pool(name="pf_hid", bufs=2) as pf_hid, \
                 tc.tile_pool(name="pf_ps", bufs=2, space="PSUM") as pf_ps, \
                 tc.tile_pool(name="pf_ps2", bufs=2, space="PSUM") as pf_ps2, \
                 tc.tile_pool(name="pf_gw", bufs=1, space="PSUM") as pf_gw:
                for t in range(NT):
                    sl = slice(t * TC, (t + 1) * TC)
                    # gate weight row for this core's expert
                    lg = pf_gw.tile([4, TC], f32, tag="lg")
                    for k in range(8):
                        nc.tensor.matmul(lg[:], lhsT=wmg_sb[:, k * 4:(k + 1) * 4],
                                         rhs=hf[k][:, sl],
                                         start=(k == 0), stop=(k == 7))
                    gx = pf.tile([4, TC], bf16, tag="gx")
                    nc.scalar.activation(gx[:], lg[:], AF.Exp)
                    sm = pf_gw.tile([1, TC], f32, tag="sm")
                    nc.tensor.matmul(sm[:], lhsT=ones4[:], rhs=gx[:],
                                     start=True, stop=True)
                    sel = pf_gw.tile([1, TC], f32, tag="sel")
                    nc.tensor.matmul(sel[:], lhsT=esel_sb[:], rhs=gx[:],
                                     start=True, stop=True)
                    rc = pf.tile([1, TC], f32, tag="rc")
                    nc.vector.reciprocal(rc[:], sm[:])
                    gwrow = pf.tile([1, TC], f32, tag="gwrow")
                    nc.vector.tensor_mul(gwrow[:], sel[:], rc[:])
                    gwb_ps = pf_gw.tile([P, TC], f32, tag="gwb_ps")
                    nc.tensor.matmul(gwb_ps[:], lhsT=ones1f[:], rhs=gwrow[:],
                                     start=True, stop=True)
                    gwb = pf.tile([P, TC], f32, tag="gwb")
                    nc.scalar.activation(gwb[:], gwb_ps[:], AF.Copy)

                    # hid = silu(w1 @ h + b1)
                    hid = [pf_hid.tile([P, TC], bf16, tag=f"hid{ft}",
                                       name=f"hid{ft}") for ft in range(16)]
                    for ft in range(16):
                        hp = pf_ps.tile([P, TC], f32, tag="hid")
                        for k in range(8):
                            nc.tensor.matmul(
                                hp[:],
                                lhsT=w1_sb[k][:, ft * P:(ft + 1) * P],
                                rhs=hf[k][:, sl], start=(k == 0), stop=(k == 7))
                        sg = pf.tile([P, TC], bf16, tag="sg")
                        nc.scalar.activation(sg[:], hp[:], AF.Sigmoid,
                                             bias=b1s_sb[:, ft:ft + 1])
                        nc.vector.scalar_tensor_tensor(
                            out=hid[ft][:], in0=hp[:], scalar=b1s_sb[:, ft:ft + 1],
                            in1=sg[:], op0=OP.add, op1=OP.mult)
                    for dt in range(8):
                        op = pf_ps2.tile([P, TC], f32, tag="out")
                        for ft in range(16):
                            nc.tensor.matmul(
                                op[:],
                                lhsT=w2_sb[ft][:, dt * P:(dt + 1) * P],
                                rhs=hid[ft][:], start=(ft == 0), stop=False,
                                skip_group_check=True)
                        nc.tensor.matmul(op[:], lhsT=b2s_sb[:, dt * P:(dt + 1) * P],
                                         rhs=ones512[:], start=False, stop=True,
                                         skip_group_check=True)
                        par = pf.tile([P, TC], bf16, tag="par")
                        nc.vector.tensor_mul(par[:], op[:], gwb[:])
                        nc.scalar.dma_start(out=rs_in[t][dt * P:(dt + 1) * P, :],
                                            in_=par[:])
                    nc.gpsimd.collective_compute(
                        "ReduceScatter", mybir.AluOpType.add, replica_groups=rg,
                        ins=[rs_in[t][:]], outs=[rs_out[t][:]])
                    fo = pf.tile([P, TC], bf16, tag="fo")
                    nc.sync.dma_start(out=fo[:], in_=rs_out[t][:])
                    yo = pf.tile([P, TC], f32, tag="yo")
                    nc.vector.tensor_add(yo[:], x2s32[:, sl], fo[:])
                    nc.scalar.dma_start(out=y_d[:, sl], in_=yo[:])
            late_stack.close()
    nc.finalize()
    return nc


def _prep_inputs(inputs):
    x = np.asarray(inputs["x"])[0]          # [T, D] f32
    w_dw = np.asarray(inputs["w_dw"])
    w_pw = np.asarray(inputs["w_pw"])
    w_qkv = np.asarray(inputs["w_qkv"])
    w_o = np.asarray(inputs["w_o"])
    w_gate = np.asarray(inputs["w_gate"])
    b_gate = np.asarray(inputs["b_gate"])
    ln_g = np.asarray(inputs["ln_g"])
    ln_b = np.asarray(inputs["ln_b"])
    w_mg = np.asarray(inputs["w_moe_gate"])
    w1 = np.asarray(inputs["w1"])
    b1 = np.asarray(inputs["b1"])
    w2 = np.asarray(inputs["w2"])
    b2 = np.asarray(inputs["b2"])

    xT = np.ascontiguousarray(x.T)                       # [D, T]
    xtp = np.zeros((D, T + 2), dtype=BF)
    xtp[:, 2:] = xT.astype(BF)

    # depthwise taps per partition: wdw[p, 3k+j] = w_dw[k*128+p, j]
    wdw = np.ascontiguousarray(
        w_dw.reshape(8, P, 3).transpose(1, 0, 2).reshape(P, 24)
    ).astype(np.float32)

    # rope tables
    inv_freq = 1.0 / (10000.0 ** (np.arange(0, DH, 2, dtype=np.float32) / DH))
    pos = np.arange(T, dtype=np.float32)
    theta = pos[None, :] * inv_freq[:, None]             # [32, T]
    cos64 = np.concatenate([np.cos(theta), np.cos(theta)], axis=0)
    sin64 = np.concatenate([-np.sin(theta), np.sin(theta)], axis=0)
    ctab = np.tile(cos64, (2, 1)).astype(BF)             # [128, T]
    stab = np.tile(sin64, (2, 1)).astype(BF)

    p64 = np.zeros((P, P), dtype=BF)
    for r in range(P):
        p64[r, (r % 64 + 32) % 64 + 64 * (r // 64)] = 1.0
    # p64 is used as lhsT: out[i,t] = sum_k p64[k,i] q[k,t] = q[swap(i),t]

    mask = np.triu(np.ones((P, P), np.float32))          # [s, t] keep s<=t

    perm = np.concatenate([np.arange(0, DH, 2), np.arange(1, DH, 2)])
    wq, wk, wv = w_qkv[0:D], w_qkv[D:2 * D], w_qkv[2 * D:3 * D]

    in_maps = []
    for c in range(NC_N):
        heads = [2 * c, 2 * c + 1]
        qrows = np.concatenate([h * DH + perm for h in heads])
        krows = qrows
        vrows = np.concatenate([np.arange(h * DH, (h + 1) * DH) for h in heads])
        wqkvT = np.concatenate(
            [wq[qrows].T, wk[krows].T, wv[vrows].T], axis=1)   # [1024, 384]
        e, hh = c // 2, c % 2
        w1s = w1[e, hh * 2048:(hh + 1) * 2048, :].T          # [1024, 2048]
        b1s = b1[e, hh * 2048:(hh + 1) * 2048]               # [2048]
        w2s = w2[e, :, hh * 2048:(hh + 1) * 2048].T          # [2048, 1024]
        b2s = (b2[e] if hh == 0 else np.zeros(D, np.float32))
        esel = np.zeros((4, 1), dtype=BF)
        esel[e, 0] = 1.0

        w2s_packed = np.empty((P, 16 * 1024), dtype=BF)
        for k in range(16):
            w2s_packed[:, k * 1024:(k + 1) * 1024] = \
                w2s[k * P:(k + 1) * P].astype(BF)
        b1sp = b1s.reshape(16, P).T.astype(np.float32)
        in_maps.append({
            "xtp": xtp,
            "xs32": np.ascontiguousarray(xT[c * P:(c + 1) * P]).astype(np.float32),
            "wdw": wdw,
            "wpw": _pack_k(w_pw.T[:, c * P:(c + 1) * P].astype(BF), P),
            "wqkv": _pack_k(wqkvT.astype(BF), 384),
            "ctab": ctab, "stab": stab, "p64": p64, "mask": mask,
            "wo": _pack_k(w_o.T[:, c * P:(c + 1) * P].astype(BF), P),
            "wg": _pack_k(w_gate.T[:, c * P:(c + 1) * P].astype(BF), P),
            "bg": b_gate[c * P:(c + 1) * P].reshape(P, 1).astype(np.float32),
            "lng": ln_g[c * P:(c + 1) * P].reshape(P, 1).astype(np.float32),
            "lnb": ln_b[c * P:(c + 1) * P].reshape(P, 1).astype(np.float32),
            "wmg": _pack_k(w_mg.T.astype(BF), 4),
            "esel": esel,
            "w1s": _pack_k(w1s.astype(BF), 2048),
            "b1s": b1sp,
            "w2s": w2s_packed,
            "b2s": b2s.reshape(1, D).astype(BF),
        })
    return in_maps


def _pack_k(mat_km, M, rot=0):
    """[1024, M] -> [128, 8*M]: k-block i holds rows of block (rot+i)%8.

    rot matches the conv out-block rotation: x1f[i] holds x1 rows
    ((rot+i)%8)*128.., so contraction block i must use those weight rows.
    """
    out = np.empty((P, 8 * M), dtype=mat_km.dtype)
    for k in range(8):
        o = (rot + k) % 8
        out[:, k * M:(k + 1) * M] = mat_km[o * P:(o + 1) * P]
    return out


def kernel(**inputs) -> np.ndarray:
    global _PROGRAM
    from concourse.bass_utils import run_bass_kernel_spmd

    if _PROGRAM is None:
        _PROGRAM = _build_program()
    nc = _PROGRAM
    in_maps = _prep_inputs(inputs)
    last_err = None
    for _attempt in range(2):
        try:
            res = run_bass_kernel_spmd(nc, in_maps, list(range(NC_N)))
            break
        except Exception as exc:  # transient device hiccups: retry once
            last_err = exc
    else:
        raise last_err
    outT = np.empty((D, T), dtype=np.float32)
    for c in range(NC_N):
        outT[c * P:(c + 1) * P] = res.results[c]["y"]
    return np.ascontiguousarray(outT.T)[None, :, :].astype(np.float32)
